# revision 21
# baseline (speedup 1.0000x reference)
"""Sliding-window GQA causal self-attention for Trainium2, 8 NeuronCores.

Sharding: 8 cores = 4 batches x 2 head-shards. Each core handles one batch
and 2 of the 4 KV groups (8 of 16 Q heads). Core computes a full [C, T]
partial of the output projection in bf16; host sums the two shards per batch.

All matmul operands are bf16 (rel err ~4e-3 total). Band masks are applied
by seeding the score PSUM region with -1e30 via identity matmuls (same
accumulation group as the scores). Softmax denominators come from a ones
column appended to the V stationary. RoPE's partition swap is a permutation
matmul; rsqrt is exp(-0.5*ln(x)) so the Act engine needs one table only.
"""
import numpy as np
import ml_dtypes

B, T, C = 4, 1024, 1024
H, HKV, D = 16, 4, 64
REP = H // HKV
WINDOW = 256
GATE_CH = 12
NCORES = 8
EPS = float(np.finfo(np.float32).eps)
QK_SCALE = 1.2 * 1.2 / 8.0
NEG = -1.0e30
BF16 = ml_dtypes.bfloat16

_CACHE = {}


def _build_program(debug=False, reps=1):
    from contextlib import ExitStack
    import concourse.bass as bass
    import concourse.tile as tile
    from concourse import bacc, mybir

    f32 = mybir.dt.float32
    bf16 = mybir.dt.bfloat16
    ts = bass.ts

    nc = bacc.Bacc("TRN2", target_bir_lowering=False, debug=False,
                   enable_asserts=True, num_devices=NCORES)

    def din(name, shape, dt=bf16):
        return nc.dram_tensor(name, shape, dt, kind="ExternalInput").ap()

    xt = din("xt", [C, T])
    wq = din("wq", [C, 512])
    wk = din("wk", [C, 128])
    wv = din("wv", [C, 128])
    wo = din("wo", [512, C])
    wgn = din("wgn", [16, 2])            # negated gate weights (16 = padded)
    vet = din("vet", [128, 8, 128])      # 3*ve, t-major: [t%128, t//128, ch]
    cosb = din("cosb", [128, T])
    sinbw = din("sinbw", [128, T])       # swap32(sin) with sign pattern
    cb16 = din("cb16", [128, 12, 128])   # consts, see _const_inputs
    cepsb = din("cepsb", [128, 1], f32)
    outT = nc.dram_tensor("out_t", [C, T], bf16, kind="ExternalOutput").ap()

    Exp = mybir.ActivationFunctionType.Exp
    Sqrt = mybir.ActivationFunctionType.Sqrt
    Copy = mybir.ActivationFunctionType.Copy
    mult = mybir.AluOpType.mult
    divide = mybir.AluOpType.divide
    add = mybir.AluOpType.add
    bypass = mybir.AluOpType.bypass

    with tile.TileContext(nc) as tc:
     for _rep in range(reps):
      with ExitStack() as ctx:
        sing = ctx.enter_context(tc.tile_pool(name="sing", bufs=1))

        # ---------- persistent tiles + input DMAs (priority order) ----------
        cb = sing.tile([128, 12, 128], bf16, name="cb")
        nc.sync.dma_start(cb[:], cb16[:])
        wk_sb = sing.tile([128, 8, 128], bf16, name="wk_sb")
        nc.sync.dma_start(wk_sb[:], wk[:])
        xt_sb = sing.tile([128, 8, T], bf16, name="xt_sb")
        nc.sync.dma_start(xt_sb[:, 0:2, :], xt[0:256, :])
        wv_sb = sing.tile([128, 8, 128], bf16, name="wv_sb")
        nc.sync.dma_start(wv_sb[:], wv[:])
        nc.sync.dma_start(xt_sb[:, 2:4, :], xt[256:512, :])
        wq_sb = sing.tile([128, 8, 512], bf16, name="wq_sb")
        nc.sync.dma_start(wq_sb[:], wq[:])
        nc.sync.dma_start(xt_sb[:, 4:6, :], xt[512:768, :])
        nc.sync.dma_start(xt_sb[:, 6:8, :], xt[768:1024, :])
        wg_sb = sing.tile([16, 2], bf16, name="wg_sb")
        nc.sync.dma_start(wg_sb[:], wgn[:])
        epsb_sb = sing.tile([128, 1], f32, name="epsb_sb")
        nc.sync.dma_start(epsb_sb[:], cepsb[:])
        cos_sb = sing.tile([128, T], bf16, name="cos_sb")
        nc.sync.dma_start(cos_sb[:], cosb[:])
        sinw_sb = sing.tile([128, T], bf16, name="sinw_sb")
        nc.sync.dma_start(sinw_sb[:], sinbw[:])
        vet_sb = sing.tile([128, 8, 128], bf16, name="vet_sb")
        nc.sync.dma_start(vet_sb[:], vet[:])
        wo_sb = sing.tile([128, 4, C], bf16, name="wo_sb")
        nc.sync.dma_start(wo_sb[:], wo[:])

        ident = cb[:, 0, :]
        pswap = cb[:, 1, :]
        tmc = cb[:, 2, :]
        tmw = cb[:, 3, :]
        indq4 = [cb[:, 4, 0:4], cb[:, 4, 4:8]]   # even r / odd r
        ind014 = [cb[0:4, 5 + r, :] for r in range(4)]
        indq2 = cb[:, 4, 32:34]

        indbk = cb[0:2, 9, :]
        indbg = cb[0:2, 10, :]
        ones128 = cb[0:1, 11, :]

        kTf = sing.tile([128, T], bf16, name="kTf")
        qTf = [sing.tile([128, T], bf16, name=f"qTf{r}") for r in range(4)]
        v_sb = sing.tile([128, 8, 130], bf16, name="v_sb")
        yTf = [sing.tile([128, T], bf16, name=f"yTf{r}") for r in range(4)]
        nc.vector.memset(v_sb[:, :, 64:65], 1.0)
        nc.vector.memset(v_sb[:, :, 129:130], 1.0)

        # ================= Stage A: projections / rope / rms / gate =========
        with tc.tile_pool(name="stA", bufs=2) as stA, \
             tc.tile_pool(name="pA1", bufs=1, space="PSUM") as pA1:

            k_ps = [pA1.tile([128, 512], f32, name=f"k_ps{h}", tag=f"kps{h}")
                    for h in range(2)]
            v_ps = [pA1.tile([128, 512], f32, name=f"v_ps{h}", tag="vps",
                             bufs=1) for h in range(2)]
            g_ps = [pA1.tile([2, 512], f32, name=f"g_ps{h}", tag="gps",
                             bufs=2) for h in range(2)]

            def tsl(h):
                return slice(512 * h, 512 * h + 512)

            # --- PE: gate projection first (tiny, frees Act/PE early),
            # then k/v chunk-interleaved with the xt DMA arrivals
            for h in range(2):
                nc.tensor.matmul(g_ps[h][:], wg_sb[:], xt_sb[0:16, 0, tsl(h)],
                                 start=True, stop=True)
            for kc in range(8):
                for h in range(2):
                    nc.tensor.matmul(k_ps[h][:], wk_sb[:, kc, :],
                                     xt_sb[:, kc, tsl(h)],
                                     start=(kc == 0), stop=(kc == 7))
                nc.tensor.matmul(v_ps[0][:], wv_sb[:, kc, :],
                                 xt_sb[:, kc, tsl(0)],
                                 start=(kc == 0), stop=(kc == 7))
            for kc in range(8):
                nc.tensor.matmul(v_ps[1][:], wv_sb[:, kc, :],
                                 xt_sb[:, kc, tsl(1)],
                                 start=(kc == 0), stop=(kc == 7))

            # gate chain: expg -> transpose to t-partition cols -> 1/(1+x)
            expg, gT = {}, None
            for h in range(2):
                expg[h] = stA.tile([2, 512], bf16, name="expg", tag="expg",
                                   bufs=2)
                nc.scalar.activation(expg[h][:], g_ps[h][:], Exp)
            gT = pA1.tile([128, 8, 2], bf16, name="gT", tag="sm", bufs=1)
            for h in range(2):
                for tb in range(4):
                    nc.tensor.transpose(gT[:, 4 * h + tb, :],
                                        expg[h][:, ts(tb, 128)],
                                        ident[0:2, 0:2])
            g1sb = stA.tile([128, 8, 2], bf16, name="g1sb", tag="g1", bufs=1)
            nc.vector.scalar_tensor_tensor(g1sb[:], gT[:], 1.0, gT[:],
                                           add, bypass)
            grec = stA.tile([128, 8, 2], bf16, name="grec", tag="grec",
                            bufs=1)
            with nc.allow_low_precision("gate recip bf16"):
                nc.vector.reciprocal(grec[:], g1sb[:])

            # k chain + v copies (Act), rope muls (Pool), squares (DVE)
            k_sb, ku, kc_, k2 = {}, {}, {}, {}
            v_raw = {}
            for h in range(2):
                k_sb[h] = stA.tile([128, 512], bf16, name="k_sb", tag="ksb",
                                   bufs=2)
                nc.scalar.activation(k_sb[h][:], k_ps[h][:], Copy)
                v_raw[h] = stA.tile([128, 512], bf16, name="v_raw", tag="vrw",
                                    bufs=2)
                nc.scalar.activation(v_raw[h][:], v_ps[h][:], Copy)
                ku[h] = stA.tile([128, 512], bf16, name="ku", tag="ku", bufs=2)
                nc.gpsimd.tensor_mul(ku[h][:], k_sb[h][:], sinw_sb[:, tsl(h)])
                kc_[h] = stA.tile([128, 512], bf16, name="kc_", tag="kc",
                                  bufs=2)
                nc.gpsimd.tensor_mul(kc_[h][:], k_sb[h][:], cos_sb[:, tsl(h)])
                k2[h] = sing.tile([128, 512], bf16, name=f"k2_{h}")
                nc.vector.tensor_mul(k2[h][:], k_sb[h][:], k_sb[h][:])

            # --- PE: v transposes, then v_sb = gate*ve + v_t (DVE stt)
            vt8 = pA1.tile([128, 8, 128], bf16, name="vt8", tag="vt", bufs=1)
            for h in range(2):
                for tb in range(4):
                    nc.tensor.transpose(vt8[:, 4 * h + tb, :],
                                        v_raw[h][:, ts(tb, 128)], ident[:])
            for jb in range(8):
                for gg in range(2):
                    nc.vector.scalar_tensor_tensor(
                        v_sb[:, jb, 65 * gg:65 * gg + 64],
                        vet_sb[:, jb, 64 * gg:64 * gg + 64],
                        grec[:, jb, gg:gg + 1],
                        vt8[:, jb, 64 * gg:64 * gg + 64], mult, add)

            # --- PE: rope swaps for k
            ksw_ps = {}
            for h in range(2):
                ksw_ps[h] = pA1.tile([128, 512], f32, name="ksw_ps",
                                     tag="sm", bufs=1)
                nc.tensor.matmul(ksw_ps[h][:], pswap[:], ku[h][:],
                                 start=True, stop=True)
            kpre = {}
            for h in range(2):
                kpre[h] = sing.tile([128, 512], bf16, name=f"kpre{h}")
                nc.vector.tensor_add(kpre[h][:], ksw_ps[h][:], kc_[h][:])

        with tc.tile_pool(name="stA2", bufs=2) as stA, \
             tc.tile_pool(name="pA2", bufs=1, space="PSUM") as pA2:
            def tsl(h):
                return slice(512 * h, 512 * h + 512)

            q_ps, msq_ps = {}, {}
            for h in range(2):
                for p in range(2):
                    msq_ps[(h, p)] = pA2.tile([4, 512], f32,
                                              name=f"msq{h}{p}",
                                              tag=f"msq{h}", bufs=1)

            qu, qc_, q2, qsw_ps, qpre = {}, {}, {}, {}, {}

            def q_elem(r, h):
                q_sb = stA.tile([128, 512], bf16, name="q_sb", tag="qsb",
                                bufs=2)
                nc.scalar.activation(q_sb[:], q_ps[(r, h)][:], Copy)
                u = stA.tile([128, 512], bf16, name="qu", tag="qu", bufs=2)
                nc.gpsimd.tensor_mul(u[:], q_sb[:], sinw_sb[:, tsl(h)])
                qu[(r, h)] = u
                c2 = stA.tile([128, 512], bf16, name="qc_", tag="qc", bufs=2)
                nc.gpsimd.tensor_mul(c2[:], q_sb[:], cos_sb[:, tsl(h)])
                qc_[(r, h)] = c2
                s2 = stA.tile([128, 512], bf16, name="q2", tag="q2", bufs=2)
                nc.vector.tensor_mul(s2[:], q_sb[:], q_sb[:])
                q2[(r, h)] = s2

            def q_pe_tail(r):
                # swap matmul + msq accumulation for row-pair of r
                for h in range(2):
                    sw = pA2.tile([128, 512], f32, name="qsw_ps", tag="qsw",
                                  bufs=1)
                    nc.tensor.matmul(sw[:], pswap[:], qu[(r, h)][:],
                                     start=True, stop=True)
                    qsw_ps[(r, h)] = sw
                    nc.tensor.matmul(msq_ps[(h, r // 2)][:], indq4[r % 2][:],
                                     q2[(r, h)][:], start=(r % 2 == 0),
                                     stop=(r % 2 == 1), skip_group_check=True)

            def q_add(r):
                for h in range(2):
                    qp = stA.tile([128, 512], bf16, name="qpre", tag="qpre",
                                  bufs=4)
                    nc.vector.tensor_add(qp[:], qsw_ps[(r, h)][:],
                                         qc_[(r, h)][:])
                    qpre[(r, h)] = qp

            def q_rms(p):
                # sqrt + divide-fold for r pair p (r = 2p, 2p+1)
                for h in range(2):
                    sq4 = stA.tile([4, 512], bf16, name="sq4", tag="sq4",
                                   bufs=2)
                    nc.scalar.activation(sq4[:], msq_ps[(h, p)][:], Sqrt,
                                         bias=epsb_sb[0:4, :])
                    for r in (2 * p, 2 * p + 1):
                        rb_ps = pA2.tile([128, 512], f32, name="rb_ps",
                                         tag="rb", bufs=1)
                        nc.tensor.matmul(rb_ps[:], ind014[r][:], sq4[:],
                                         start=True, stop=True)
                        nc.vector.tensor_tensor(qTf[r][:, tsl(h)],
                                                qpre[(r, h)][:], rb_ps[:],
                                                divide)

            def q_proj(r):
                for h in range(2):
                    qp = pA2.tile([128, 512], f32, name=f"q_ps{r}{h}",
                                  tag="qps", bufs=2)
                    for kc in range(8):
                        nc.tensor.matmul(qp[:], wq_sb[:, kc, ts(r, 128)],
                                         xt_sb[:, kc, tsl(h)],
                                         start=(kc == 0), stop=(kc == 7))
                    q_ps[(r, h)] = qp
                    q_elem(r, h)

            # k-rms (PE parts woven between q projections)
            msk_ps, rkb_ps, sqk = {}, {}, {}
            q_proj(0)
            for h in range(2):
                msk_ps[h] = pA2.tile([2, 512], f32, name="msk_ps", tag="msk",
                                     bufs=1)
                nc.tensor.matmul(msk_ps[h][:], indq2[:], k2[h][:],
                                 start=True, stop=True)
                sqk[h] = stA.tile([2, 512], bf16, name="sqk", tag="sqk",
                                  bufs=2)
                nc.scalar.activation(sqk[h][:], msk_ps[h][:], Sqrt,
                                     bias=epsb_sb[0:2, :])
            q_proj(1)
            q_pe_tail(0)
            for h in range(2):
                rkb_ps[h] = pA2.tile([128, 512], f32, name="rkb_ps",
                                     tag="rkb", bufs=1)
                nc.tensor.matmul(rkb_ps[h][:], indbk[:], sqk[h][:],
                                 start=True, stop=True)
                nc.vector.tensor_tensor(kTf[:, tsl(h)], kpre[h][:],
                                        rkb_ps[h][:], divide)
            q_add(0)
            q_proj(2)
            q_pe_tail(1)
            q_add(1)
            q_rms(0)          # qTf r0, r1 finalized here
            q_proj(3)
            q_pe_tail(2)
            q_add(2)
            q_pe_tail(3)
            q_add(3)
            q_rms(1)

        # ================= Stage B: attention ================================
        with tc.tile_pool(name="stB", bufs=2) as stB, \
             tc.tile_pool(name="pB_", bufs=1, space="PSUM") as pB_:
            for r in range(4):
                pkeep = {j: stB.tile([128, 2, 3, 128], bf16, name=f"pk{j}",
                                     tag=f"pk{j}", bufs=2) for j in (2, 3)}
                for h in range(2):
                    hsl = slice(512 * h, 512 * h + 512)
                    y_ps = pB_.tile([65, 2, 512], f32, name="y_ps",
                                    tag="yps", bufs=2)
                    jlist = list(range(0, 4)) if h == 0 else list(range(2, 8))
                    first = True
                    for j in jlist:
                        w = min(384, T - 128 * j)
                        ns = w // 128          # number of 128-wide segments
                        fresh = not (h == 1 and j in (2, 3))
                        if not fresh:
                            p2 = pkeep[j]      # cached from h == 0
                        else:
                            p2 = pkeep.get(j)
                            if p2 is None:
                                p2 = stB.tile([128, 2, 3, 128], bf16,
                                              name="p2", tag="p2", bufs=3)
                            sc2 = pB_.tile([128, 2, 3, 128], f32, name="sc2",
                                           tag="sc", bufs=2)
                            for gg in range(2):
                                dsl = slice(64 * gg, 64 * gg + 64)
                                qsl0 = slice(128 * j, 128 * j + 128)
                                # diag segment: mask seed + score, one group
                                nc.tensor.matmul(
                                    sc2[:, gg, 0, :], ident, tmc,
                                    start=True, stop=False)
                                nc.tensor.matmul(
                                    sc2[:, gg, 0, :], kTf[dsl, qsl0],
                                    qTf[r][dsl, qsl0], start=False, stop=True,
                                    skip_group_check=True)
                                if ns > 1:    # mid segment: no mask
                                    qsl1 = slice(128 * j + 128, 128 * j + 256)
                                    nc.tensor.matmul(
                                        sc2[:, gg, 1, :],
                                        kTf[dsl, qsl0], qTf[r][dsl, qsl1],
                                        start=True, stop=True,
                                        skip_group_check=True)
                                if ns > 2:    # window segment: seed + score
                                    qsl2 = slice(128 * j + 256, 128 * j + 384)
                                    nc.tensor.matmul(
                                        sc2[:, gg, 2, :], ident, tmw,
                                        start=True, stop=False,
                                        skip_group_check=True)
                                    nc.tensor.matmul(
                                        sc2[:, gg, 2, :], kTf[dsl, qsl0],
                                        qTf[r][dsl, qsl2], start=False,
                                        stop=True, skip_group_check=True)
                            nc.scalar.activation(p2[:, :, 0:ns, :],
                                                 sc2[:, :, 0:ns, :], Exp)
                        a = max(128 * j, 512 * h)
                        b = min(128 * j + w, 512 * h + 512)
                        s0, s1 = (a - 128 * j) // 128, (b - 128 * j) // 128
                        for gg in range(2):
                            nc.tensor.matmul(
                                y_ps[:, gg, a - 512 * h:b - 512 * h],
                                v_sb[:, j, 65 * gg:65 * gg + 65],
                                p2[:, gg, s0:s1, :],
                                start=first, stop=(j == jlist[-1]),
                                skip_group_check=True)
                        first = False
                    # normalize: 1/sums, broadcast via ones matmul, 2 muls
                    rsum = stB.tile([1, 2, 512], bf16, name="rsum",
                                    tag="rsum", bufs=2)
                    with nc.allow_low_precision("1/sums bf16"):
                        nc.vector.reciprocal(rsum[:], y_ps[64:65, :, :])
                    rbs_ps = pB_.tile([128, 512], f32, name="rbs_ps",
                                      tag="sc", bufs=2)
                    nc.tensor.matmul(rbs_ps[0:64, :], ones128[:, 0:64],
                                     rsum[:, 0, :], start=True, stop=True)
                    nc.tensor.matmul(rbs_ps[64:128, :], ones128[:, 0:64],
                                     rsum[:, 1, :], start=True, stop=True,
                                     skip_group_check=True)
                    rbs_sb = stB.tile([128, 512], bf16, name="rbs_sb",
                                      tag="rbs_sb", bufs=2)
                    nc.scalar.activation(rbs_sb[:], rbs_ps[:], Copy)
                    for gg in range(2):
                        nc.vector.tensor_mul(yTf[r][ts(gg, 64), hsl],
                                             y_ps[0:64, gg, :],
                                             rbs_sb[ts(gg, 64), :])

        # ================= Stage C: output projection ========================
        with tc.tile_pool(name="stC", bufs=3) as stC, \
             tc.tile_pool(name="pC_", bufs=2, space="PSUM") as pC_:
            for h in range(2):
                hsl = slice(512 * h, 512 * h + 512)
                for ct in range(8):
                    o_ps = pC_.tile([128, 512], f32, name="o_ps", tag="ops")
                    for kr in range(4):
                        nc.tensor.matmul(o_ps[:], wo_sb[:, kr, ts(ct, 128)],
                                         yTf[kr][:, hsl], start=(kr == 0),
                                         stop=(kr == 3))
                    o_sb = stC.tile([128, 512], bf16, name="o_sb", tag="osb")
                    if ct % 2 == 0:
                        nc.vector.tensor_copy(o_sb[:], o_ps[:])
                    else:
                        nc.scalar.activation(o_sb[:], o_ps[:], Copy)
                    nc.sync.dma_start(outT[ts(ct, 128), hsl], o_sb[:])

    nc.compile()
    return nc


def _const_inputs():
    cb = np.zeros((128, 12, 128), dtype=np.float32)
    # 0: identity
    cb[:, 0, :] = np.eye(128, dtype=np.float32)
    # 1: pswap  P[c, m] = 1 iff c == swap(m), swap = +-32 within 64-block
    m = np.arange(128)
    sw = np.where((m % 64) < 32, m + 32, m - 32)
    cb[sw, 1, m] = 1.0
    # 2: Tc diag mask (keep qcol >= kpos), 3: Tw window mask (keep qcol <= kpos)
    p = np.arange(128)[:, None]
    c = np.arange(128)[None, :]
    cb[:, 2, :] = np.where(c >= p, 0.0, NEG)
    cb[:, 3, :] = np.where(c <= p, 0.0, NEG)
    # 4: cols 0:4 = stationary for even r (out rows 0:2 of msq4),
    #    cols 4:8 = odd r (out rows 2:4), cols 32:34 indq2 (k)
    cb[0:64, 4, 0] = 1.0 / D
    cb[64:128, 4, 1] = 1.0 / D
    cb[0:64, 4, 6] = 1.0 / D
    cb[64:128, 4, 7] = 1.0 / D
    cb[0:64, 4, 32] = 1.0 / D
    cb[64:128, 4, 33] = 1.0 / D
    # 5..8: ind014 per r: rsq4 row (2*(r%2)+gg) -> out gg rows, val 1/QK_SCALE
    for r in range(4):
        i = r % 2
        cb[2 * i, 5 + r, 0:64] = 1.0 / QK_SCALE
        cb[2 * i + 1, 5 + r, 64:128] = 1.0 / QK_SCALE
    # 9: indbk rows 0:2 (1/1.2), 10: indbg rows 0:2 (1.0), 11: ones row 0
    cb[0, 9, 0:64] = 1.0 / 1.2
    cb[1, 9, 64:128] = 1.0 / 1.2
    cb[0, 10, 0:64] = 1.0
    cb[1, 10, 64:128] = 1.0
    cb[0, 11, :] = 1.0
    epsb = np.full((128, 1), EPS, dtype=np.float32)
    return dict(cb16=cb.astype(BF16), cepsb=epsb)


def _prep_core_inputs(x, ve3, cosb, sinbw, Wq, Wk, Wv, Wo, Wg, consts, b, s):
    g0, g1 = 2 * s, 2 * s + 1
    bf = lambda a: np.ascontiguousarray(a).astype(BF16)
    xt = bf(x[b].T)

    Wq4 = Wq.reshape(HKV, REP, D, C)
    wq_rows = np.concatenate([Wq4[g, r] for r in range(REP) for g in (g0, g1)],
                             axis=0)                       # (512, C)
    wq = bf(wq_rows.T)                                     # (C, 512)
    Wk3 = Wk.reshape(HKV, D, C)
    wk = bf(np.concatenate([Wk3[g0], Wk3[g1]], axis=0).T)
    Wv3 = Wv.reshape(HKV, D, C)
    wv = bf(np.concatenate([Wv3[g0], Wv3[g1]], axis=0).T)

    Wo4 = Wo.reshape(C, HKV, REP, D)
    wo_cols = np.concatenate([Wo4[:, g, r, :] for r in range(REP)
                              for g in (g0, g1)], axis=1)  # (C, 512)
    wo = bf(wo_cols.T)                                     # (512, C)

    wgn = np.zeros((16, 2), dtype=np.float32)
    wgn[0:GATE_CH, 0] = -Wg[g0]
    wgn[0:GATE_CH, 1] = -Wg[g1]

    ve4 = ve3[b].reshape(T, HKV, D)
    vet2 = np.concatenate([ve4[:, g0, :], ve4[:, g1, :]], axis=1)  # (T, 128)
    vet = bf(vet2.reshape(8, 128, 128).transpose(1, 0, 2))  # (128, 8, 128)

    d = dict(xt=xt, wq=wq, wk=wk, wv=wv, wo=wo, wgn=wgn.astype(BF16),
             vet=vet, cosb=cosb, sinbw=sinbw)
    d.update(consts)
    return d


def kernel(x, ve, cos, sin, Wq, Wk, Wv, Wo, Wg, window_size):
    from concourse.bass_utils import run_bass_kernel_spmd

    assert int(window_size) == WINDOW
    x = np.asarray(x, dtype=np.float32)
    ve = np.asarray(ve, dtype=np.float32)
    Wq = np.asarray(Wq, dtype=np.float32)
    Wk = np.asarray(Wk, dtype=np.float32)
    Wv = np.asarray(Wv, dtype=np.float32)
    Wo = np.asarray(Wo, dtype=np.float32)
    Wg = np.asarray(Wg, dtype=np.float32)
    c = np.asarray(cos, dtype=np.float32).reshape(T, D // 2)   # (T, 32)
    sn = np.asarray(sin, dtype=np.float32).reshape(T, D // 2)

    cosb = np.ascontiguousarray(np.tile(c.T, (4, 1))).astype(BF16)
    sinbw = np.ascontiguousarray(
        np.concatenate([-sn.T, sn.T, -sn.T, sn.T], axis=0)).astype(BF16)
    ve3 = 3.0 * ve
    consts = _const_inputs()

    if "nc" not in _CACHE:
        _CACHE["nc"] = _build_program()
    nc = _CACHE["nc"]

    in_maps = []
    for core in range(NCORES):
        b, s = core // 2, core % 2
        in_maps.append(_prep_core_inputs(x, ve3, cosb, sinbw,
                                         Wq, Wk, Wv, Wo, Wg, consts, b, s))

    res = run_bass_kernel_spmd(nc, in_maps, core_ids=list(range(NCORES)))
    out = np.empty((B, T, C), dtype=np.float32)
    for b in range(B):
        acc = (res.results[2 * b]["out_t"].astype(np.float32)
               + res.results[2 * b + 1]["out_t"].astype(np.float32))
        out[b] = acc.T
    return out


# revision 26
# speedup vs baseline: 1.1214x; 1.1214x over previous
"""Sliding-window GQA causal self-attention for Trainium2, 8 NeuronCores.

Sharding: 8 cores = 4 batches x 2 head-shards. Each core handles one batch
and 2 of the 4 KV groups (8 of 16 Q heads). Core computes a full [C, T]
partial of the output projection in bf16; host sums the two shards per batch.

All matmul operands are bf16 (rel err ~4e-3 total). Band masks are applied
by seeding the score PSUM region with -1e30 via identity matmuls (same
accumulation group as the scores). Softmax denominators come from a ones
column appended to the V stationary. RoPE's partition swap is a permutation
matmul; rsqrt is exp(-0.5*ln(x)) so the Act engine needs one table only.
"""
import numpy as np
import ml_dtypes

B, T, C = 4, 1024, 1024
H, HKV, D = 16, 4, 64
REP = H // HKV
WINDOW = 256
GATE_CH = 12
NCORES = 8
EPS = float(np.finfo(np.float32).eps)
QK_SCALE = 1.2 * 1.2 / 8.0
NEG = -1.0e30
BF16 = ml_dtypes.bfloat16

_CACHE = {}


def _build_program(debug=False, reps=1):
    from contextlib import ExitStack
    import concourse.bass as bass
    import concourse.tile as tile
    from concourse import bacc, mybir

    f32 = mybir.dt.float32
    bf16 = mybir.dt.bfloat16
    ts = bass.ts

    nc = bacc.Bacc("TRN2", target_bir_lowering=False, debug=False,
                   enable_asserts=True, num_devices=NCORES)

    def din(name, shape, dt=bf16):
        return nc.dram_tensor(name, shape, dt, kind="ExternalInput").ap()

    xt = din("xt", [C, T])
    wqa = din("wqa", [C, 256])
    wqb = din("wqb", [C, 256])
    wk = din("wk", [C, 128])
    wv = din("wv", [C, 128])
    wo = din("wo", [512, C])
    wgn = din("wgn", [16, 2])            # negated gate weights (16 = padded)
    vet = din("vet", [128, 8, 128])      # 3*ve, t-major: [t%128, t//128, ch]
    cosb = din("cosb", [128, T])
    sinbw = din("sinbw", [128, T])       # swap32(sin) with sign pattern
    cb16 = din("cb16", [128, 12, 128])   # consts, see _const_inputs
    cepsb = din("cepsb", [128, 1], f32)
    outT = nc.dram_tensor("out_t", [C, T], bf16, kind="ExternalOutput").ap()

    Exp = mybir.ActivationFunctionType.Exp
    Sqrt = mybir.ActivationFunctionType.Sqrt
    Copy = mybir.ActivationFunctionType.Copy
    mult = mybir.AluOpType.mult
    divide = mybir.AluOpType.divide
    add = mybir.AluOpType.add
    bypass = mybir.AluOpType.bypass

    with tile.TileContext(nc) as tc:
     for _rep in range(reps):
      with ExitStack() as ctx:
        sing = ctx.enter_context(tc.tile_pool(name="sing", bufs=1))

        # ---------- persistent tiles + input DMAs (priority order) ----------
        cb = sing.tile([128, 12, 128], bf16, name="cb")
        nc.sync.dma_start(cb[:], cb16[:])
        wk_sb = sing.tile([128, 8, 128], bf16, name="wk_sb")
        nc.sync.dma_start(wk_sb[:], wk[:])
        wg_sb = sing.tile([16, 2], bf16, name="wg_sb")
        nc.sync.dma_start(wg_sb[:], wgn[:])
        xt_sb = sing.tile([128, 8, T], bf16, name="xt_sb")
        nc.sync.dma_start(xt_sb[:, 0:2, :], xt[0:256, :])
        wv_sb = sing.tile([128, 8, 128], bf16, name="wv_sb")
        nc.sync.dma_start(wv_sb[:], wv[:])
        wq_sb = sing.tile([128, 8, 512], bf16, name="wq_sb")
        nc.sync.dma_start(wq_sb[:, :, 0:256], wqa[:])
        nc.sync.dma_start(xt_sb[:, 2:4, :], xt[256:512, :])
        nc.sync.dma_start(xt_sb[:, 4:6, :], xt[512:768, :])
        nc.sync.dma_start(xt_sb[:, 6:8, :], xt[768:1024, :])
        nc.sync.dma_start(wq_sb[:, :, 256:512], wqb[:])
        epsb_sb = sing.tile([128, 1], f32, name="epsb_sb")
        nc.sync.dma_start(epsb_sb[:], cepsb[:])
        cos_sb = sing.tile([128, T], bf16, name="cos_sb")
        nc.sync.dma_start(cos_sb[:], cosb[:])
        sinw_sb = sing.tile([128, T], bf16, name="sinw_sb")
        nc.sync.dma_start(sinw_sb[:], sinbw[:])
        vet_sb = sing.tile([128, 8, 128], bf16, name="vet_sb")
        nc.sync.dma_start(vet_sb[:], vet[:])
        wo_sb = sing.tile([128, 4, C], bf16, name="wo_sb")
        nc.sync.dma_start(wo_sb[:], wo[:])

        ident = cb[:, 0, :]
        pswap = cb[:, 1, :]
        tmc = cb[:, 2, :]
        tmw = cb[:, 3, :]
        indq4 = [cb[:, 4, 0:4], cb[:, 4, 4:8]]   # even r / odd r
        ind014 = [cb[0:4, 5 + r, :] for r in range(4)]
        indq2 = cb[:, 4, 32:34]

        indbk = cb[0:2, 9, :]
        indbg = cb[0:2, 10, :]
        ones128 = cb[0:1, 11, :]

        kTf = sing.tile([128, T], bf16, name="kTf")
        qTf = [sing.tile([128, T], bf16, name=f"qTf{r}") for r in range(4)]
        v_sb = sing.tile([128, 8, 130], bf16, name="v_sb")
        yTf = [sing.tile([128, T], bf16, name=f"yTf{r}") for r in range(4)]
        nc.vector.memset(v_sb[:, :, 64:65], 1.0)
        nc.vector.memset(v_sb[:, :, 129:130], 1.0)

        # ================= Stage A: projections / rope / rms / gate =========
        with tc.tile_pool(name="stA", bufs=2) as stA, \
             tc.tile_pool(name="pA1", bufs=1, space="PSUM") as pA1:

            k_ps = [pA1.tile([128, 512], f32, name=f"k_ps{h}", tag=f"kps{h}")
                    for h in range(2)]
            v_ps = [pA1.tile([128, 512], f32, name=f"v_ps{h}", tag="vps",
                             bufs=1) for h in range(2)]
            g_ps = [pA1.tile([2, 512], f32, name=f"g_ps{h}", tag="gps",
                             bufs=1) for h in range(2)]

            def tsl(h):
                return slice(512 * h, 512 * h + 512)

            # --- PE: gate projection first (tiny, frees Act/PE early),
            # then k/v chunk-interleaved with the xt DMA arrivals
            for h in range(2):
                nc.tensor.matmul(g_ps[h][:], wg_sb[:], xt_sb[0:16, 0, tsl(h)],
                                 start=True, stop=True)
            for kc in range(8):
                for h in range(2):
                    nc.tensor.matmul(k_ps[h][:], wk_sb[:, kc, :],
                                     xt_sb[:, kc, tsl(h)],
                                     start=(kc == 0), stop=(kc == 7))
                nc.tensor.matmul(v_ps[0][:], wv_sb[:, kc, :],
                                 xt_sb[:, kc, tsl(0)],
                                 start=(kc == 0), stop=(kc == 7))
            for kc in range(8):
                nc.tensor.matmul(v_ps[1][:], wv_sb[:, kc, :],
                                 xt_sb[:, kc, tsl(1)],
                                 start=(kc == 0), stop=(kc == 7))

            # gate chain: expg -> transpose to t-partition cols -> 1/(1+x)
            expg, gT = {}, None
            for h in range(2):
                expg[h] = stA.tile([2, 512], bf16, name="expg", tag="expg",
                                   bufs=2)
                nc.scalar.activation(expg[h][:], g_ps[h][:], Exp)
            gT = pA1.tile([128, 8, 2], bf16, name="gT", tag="sm", bufs=1)
            for h in range(2):
                for tb in range(4):
                    nc.tensor.transpose(gT[:, 4 * h + tb, :],
                                        expg[h][:, ts(tb, 128)],
                                        ident[0:2, 0:2])
            g1sb = stA.tile([128, 8, 2], bf16, name="g1sb", tag="g1", bufs=1)
            nc.vector.scalar_tensor_tensor(g1sb[:], gT[:], 1.0, gT[:],
                                           add, bypass)
            grec = stA.tile([128, 8, 2], bf16, name="grec", tag="grec",
                            bufs=1)
            with nc.allow_low_precision("gate recip bf16"):
                nc.vector.reciprocal(grec[:], g1sb[:])

            # k chain + v copies (Act), rope muls (Pool), squares (DVE)
            k_sb, ku, kc_, k2 = {}, {}, {}, {}
            v_raw = {}
            for h in range(2):
                k_sb[h] = stA.tile([128, 512], bf16, name="k_sb", tag="ksb",
                                   bufs=2)
                nc.scalar.activation(k_sb[h][:], k_ps[h][:], Copy)
                v_raw[h] = stA.tile([128, 512], bf16, name="v_raw", tag="vrw",
                                    bufs=2)
                nc.scalar.activation(v_raw[h][:], v_ps[h][:], Copy)
                ku[h] = sing.tile([128, 512], bf16, name=f"ku{h}")
                nc.gpsimd.tensor_mul(ku[h][:], k_sb[h][:], sinw_sb[:, tsl(h)])
                kc_[h] = sing.tile([128, 512], bf16, name=f"kc{h}")
                nc.gpsimd.tensor_mul(kc_[h][:], k_sb[h][:], cos_sb[:, tsl(h)])
                k2[h] = sing.tile([128, 512], bf16, name=f"k2_{h}")
                nc.vector.tensor_mul(k2[h][:], k_sb[h][:], k_sb[h][:])

            # --- PE: v transposes, then v_sb = gate*ve + v_t (DVE stt)
            vt8 = pA1.tile([128, 8, 128], bf16, name="vt8", tag="vt", bufs=1)
            for h in range(2):
                for tb in range(4):
                    nc.tensor.transpose(vt8[:, 4 * h + tb, :],
                                        v_raw[h][:, ts(tb, 128)], ident[:])
            for jb in range(8):
                for gg in range(2):
                    nc.vector.scalar_tensor_tensor(
                        v_sb[:, jb, 65 * gg:65 * gg + 64],
                        vet_sb[:, jb, 64 * gg:64 * gg + 64],
                        grec[:, jb, gg:gg + 1],
                        vt8[:, jb, 64 * gg:64 * gg + 64], mult, add)

            # (k rope swap happens in pool 2)
            # --- q projections + psum->sbuf copy + rope muls + squares
            # (allocated in pool 1 so they overlap the xt DMA phase)
            q_ps, q_sbs = {}, {}
            for r in range(4):
                for h in range(2):
                    qp = pA1.tile([128, 512], f32, name=f"q_ps{r}{h}",
                                  tag="qps", bufs=2)
                    for kc in range(8):
                        nc.tensor.matmul(qp[:], wq_sb[:, kc, ts(r, 128)],
                                         xt_sb[:, kc, tsl(h)],
                                         start=(kc == 0), stop=(kc == 7))
                    q_ps[(r, h)] = qp
                    qs = sing.tile([128, 512], bf16, name=f"q_sb{r}{h}")
                    nc.scalar.activation(qs[:], qp[:], Copy)
                    q_sbs[(r, h)] = qs

        with tc.tile_pool(name="stA2", bufs=2) as stA, \
             tc.tile_pool(name="pA2", bufs=1, space="PSUM") as pA2:
            def tsl(h):
                return slice(512 * h, 512 * h + 512)

            # --- k rope swap + rms-fold (divide), k-ms
            ksw_ps, kpre = {}, {}
            for h in range(2):
                ksw_ps[h] = pA2.tile([128, 512], f32, name="ksw_ps",
                                     tag="sw", bufs=2)
                nc.tensor.matmul(ksw_ps[h][:], pswap[:], ku[h][:],
                                 start=True, stop=True)
                kpre[h] = stA.tile([128, 512], bf16, name=f"kpre{h}",
                                   tag="kpre", bufs=2)
                nc.vector.tensor_add(kpre[h][:], ksw_ps[h][:], kc_[h][:])
            msk_ps, sqk = {}, {}
            for h in range(2):
                msk_ps[h] = pA2.tile([2, 512], f32, name="msk_ps", tag="msk",
                                     bufs=1)
                nc.tensor.matmul(msk_ps[h][:], indq2[:], k2[h][:],
                                 start=True, stop=True)
                sqk[h] = stA.tile([2, 512], bf16, name="sqk", tag="sqk",
                                  bufs=2)
                nc.scalar.activation(sqk[h][:], msk_ps[h][:], Sqrt,
                                     bias=epsb_sb[0:2, :])
            for h in range(2):
                rkb_ps = pA2.tile([128, 512], f32, name="rkb_ps",
                                  tag="bc", bufs=2)
                nc.tensor.matmul(rkb_ps[:], indbk[:], sqk[h][:],
                                 start=True, stop=True)
                nc.vector.tensor_tensor(kTf[:, tsl(h)], kpre[h][:],
                                        rkb_ps[:], divide)

            # --- q rope + rms, r-major, divide-fold
            msq_ps = {}

            for r in range(4):
                qu, qc2, q2s, qsw = {}, {}, {}, {}
                for h in range(2):
                    qs = q_sbs[(r, h)]
                    u = stA.tile([128, 512], bf16, name="qu", tag="qu",
                                 bufs=2)
                    nc.gpsimd.tensor_mul(u[:], qs[:], sinw_sb[:, tsl(h)])
                    qu[h] = u
                    c2 = stA.tile([128, 512], bf16, name="qc_", tag="qc",
                                  bufs=2)
                    nc.gpsimd.tensor_mul(c2[:], qs[:], cos_sb[:, tsl(h)])
                    qc2[h] = c2
                    s2 = stA.tile([128, 512], bf16, name="q2", tag="q2",
                                  bufs=2)
                    nc.vector.tensor_mul(s2[:], qs[:], qs[:])
                    q2s[h] = s2
                for h in range(2):
                    sw = pA2.tile([128, 512], f32, name="qsw_ps", tag="sw",
                                  bufs=2)
                    nc.tensor.matmul(sw[:], pswap[:], qu[h][:],
                                     start=True, stop=True)
                    qsw[h] = sw
                    if r % 2 == 0:
                        msq_ps[(h, r // 2)] = pA2.tile(
                            [4, 512], f32, name=f"msq{h}", tag=f"msq{h}",
                            bufs=1)
                    nc.tensor.matmul(msq_ps[(h, r // 2)][:], indq4[r % 2][:],
                                     q2s[h][:], start=(r % 2 == 0),
                                     stop=(r % 2 == 1), skip_group_check=True)
                qpre = {}
                for h in range(2):
                    qp = stA.tile([128, 512], bf16, name="qpre", tag="qpre",
                                  bufs=4)
                    nc.vector.tensor_add(qp[:], qsw[h][:], qc2[h][:])
                    qpre[(r, h)] = qp
                globals().setdefault("_qpre_all", {}).update(qpre)
                if r % 2 == 1:
                    p = r // 2
                    for h in range(2):
                        sq4 = stA.tile([4, 512], bf16, name="sq4", tag="sq4",
                                       bufs=2)
                        nc.scalar.activation(sq4[:], msq_ps[(h, p)][:], Sqrt,
                                             bias=epsb_sb[0:4, :])
                        for rr in (2 * p, 2 * p + 1):
                            rb_ps = pA2.tile([128, 512], f32, name="rb_ps",
                                             tag="bc", bufs=2)
                            nc.tensor.matmul(rb_ps[:], ind014[rr][:], sq4[:],
                                             start=True, stop=True)
                            nc.vector.tensor_tensor(
                                qTf[rr][:, tsl(h)],
                                globals()["_qpre_all"][(rr, h)][:],
                                rb_ps[:], divide)

        # ================= Stage B: attention ================================
        with tc.tile_pool(name="stB", bufs=2) as stB, \
             tc.tile_pool(name="pB_", bufs=1, space="PSUM") as pB_:
            for r in range(4):
                pkeep = {j: stB.tile([128, 2, 3, 128], bf16, name=f"pk{j}",
                                     tag=f"pk{j}", bufs=2) for j in (2, 3)}
                for h in range(2):
                    hsl = slice(512 * h, 512 * h + 512)
                    y_ps = pB_.tile([65, 2, 512], f32, name="y_ps",
                                    tag="yps", bufs=2)
                    jlist = list(range(0, 4)) if h == 0 else list(range(2, 8))
                    first = True
                    for j in jlist:
                        w = min(384, T - 128 * j)
                        ns = w // 128          # number of 128-wide segments
                        fresh = not (h == 1 and j in (2, 3))
                        if not fresh:
                            p2 = pkeep[j]      # cached from h == 0
                        else:
                            p2 = pkeep.get(j)
                            if p2 is None:
                                p2 = stB.tile([128, 2, 3, 128], bf16,
                                              name="p2", tag="p2", bufs=3)
                            sc2 = pB_.tile([128, 2, 3, 128], f32, name="sc2",
                                           tag="sc", bufs=2)
                            for gg in range(2):
                                dsl = slice(64 * gg, 64 * gg + 64)
                                qsl0 = slice(128 * j, 128 * j + 128)
                                # diag segment: mask seed + score, one group
                                nc.tensor.matmul(
                                    sc2[:, gg, 0, :], ident, tmc,
                                    start=True, stop=False)
                                nc.tensor.matmul(
                                    sc2[:, gg, 0, :], kTf[dsl, qsl0],
                                    qTf[r][dsl, qsl0], start=False, stop=True,
                                    skip_group_check=True)
                                if ns > 1:    # mid segment: no mask
                                    qsl1 = slice(128 * j + 128, 128 * j + 256)
                                    nc.tensor.matmul(
                                        sc2[:, gg, 1, :],
                                        kTf[dsl, qsl0], qTf[r][dsl, qsl1],
                                        start=True, stop=True,
                                        skip_group_check=True)
                                if ns > 2:    # window segment: seed + score
                                    qsl2 = slice(128 * j + 256, 128 * j + 384)
                                    nc.tensor.matmul(
                                        sc2[:, gg, 2, :], ident, tmw,
                                        start=True, stop=False,
                                        skip_group_check=True)
                                    nc.tensor.matmul(
                                        sc2[:, gg, 2, :], kTf[dsl, qsl0],
                                        qTf[r][dsl, qsl2], start=False,
                                        stop=True, skip_group_check=True)
                            nc.scalar.activation(p2[:, :, 0:ns, :],
                                                 sc2[:, :, 0:ns, :], Exp)
                        a = max(128 * j, 512 * h)
                        b = min(128 * j + w, 512 * h + 512)
                        s0, s1 = (a - 128 * j) // 128, (b - 128 * j) // 128
                        for gg in range(2):
                            nc.tensor.matmul(
                                y_ps[:, gg, a - 512 * h:b - 512 * h],
                                v_sb[:, j, 65 * gg:65 * gg + 65],
                                p2[:, gg, s0:s1, :],
                                start=first, stop=(j == jlist[-1]),
                                skip_group_check=True)
                        first = False
                    # normalize: 1/sums, broadcast via ones matmul, 2 muls
                    rsum = stB.tile([1, 2, 512], bf16, name="rsum",
                                    tag="rsum", bufs=2)
                    with nc.allow_low_precision("1/sums bf16"):
                        nc.vector.reciprocal(rsum[:], y_ps[64:65, :, :])
                    rbs_ps = pB_.tile([128, 512], f32, name="rbs_ps",
                                      tag="sc", bufs=2)
                    nc.tensor.matmul(rbs_ps[0:64, :], ones128[:, 0:64],
                                     rsum[:, 0, :], start=True, stop=True)
                    nc.tensor.matmul(rbs_ps[64:128, :], ones128[:, 0:64],
                                     rsum[:, 1, :], start=True, stop=True,
                                     skip_group_check=True)
                    rbs_sb = stB.tile([128, 512], bf16, name="rbs_sb",
                                      tag="rbs_sb", bufs=2)
                    nc.scalar.activation(rbs_sb[:], rbs_ps[:], Copy)
                    for gg in range(2):
                        nc.vector.tensor_mul(yTf[r][ts(gg, 64), hsl],
                                             y_ps[0:64, gg, :],
                                             rbs_sb[ts(gg, 64), :])

        # ================= Stage C: output projection ========================
        with tc.tile_pool(name="stC", bufs=3) as stC, \
             tc.tile_pool(name="pC_", bufs=2, space="PSUM") as pC_:
            for h in range(2):
                hsl = slice(512 * h, 512 * h + 512)
                for ct in range(8):
                    o_ps = pC_.tile([128, 512], f32, name="o_ps", tag="ops")
                    for kr in range(4):
                        nc.tensor.matmul(o_ps[:], wo_sb[:, kr, ts(ct, 128)],
                                         yTf[kr][:, hsl], start=(kr == 0),
                                         stop=(kr == 3))
                    o_sb = stC.tile([128, 512], bf16, name="o_sb", tag="osb")
                    if ct % 2 == 0:
                        nc.vector.tensor_copy(o_sb[:], o_ps[:])
                    else:
                        nc.scalar.activation(o_sb[:], o_ps[:], Copy)
                    nc.sync.dma_start(outT[ts(ct, 128), hsl], o_sb[:])

    nc.compile()
    return nc


def _const_inputs():
    cb = np.zeros((128, 12, 128), dtype=np.float32)
    # 0: identity
    cb[:, 0, :] = np.eye(128, dtype=np.float32)
    # 1: pswap  P[c, m] = 1 iff c == swap(m), swap = +-32 within 64-block
    m = np.arange(128)
    sw = np.where((m % 64) < 32, m + 32, m - 32)
    cb[sw, 1, m] = 1.0
    # 2: Tc diag mask (keep qcol >= kpos), 3: Tw window mask (keep qcol <= kpos)
    p = np.arange(128)[:, None]
    c = np.arange(128)[None, :]
    cb[:, 2, :] = np.where(c >= p, 0.0, NEG)
    cb[:, 3, :] = np.where(c <= p, 0.0, NEG)
    # 4: cols 0:4 = stationary for even r (out rows 0:2 of msq4),
    #    cols 4:8 = odd r (out rows 2:4), cols 32:34 indq2 (k)
    cb[0:64, 4, 0] = 1.0 / D
    cb[64:128, 4, 1] = 1.0 / D
    cb[0:64, 4, 6] = 1.0 / D
    cb[64:128, 4, 7] = 1.0 / D
    cb[0:64, 4, 32] = 1.0 / D
    cb[64:128, 4, 33] = 1.0 / D
    # 5..8: ind014 per r: rsq4 row (2*(r%2)+gg) -> out gg rows, val 1/QK_SCALE
    for r in range(4):
        i = r % 2
        cb[2 * i, 5 + r, 0:64] = 1.0 / QK_SCALE
        cb[2 * i + 1, 5 + r, 64:128] = 1.0 / QK_SCALE
    # 9: indbk rows 0:2 (1/1.2), 10: indbg rows 0:2 (1.0), 11: ones row 0
    cb[0, 9, 0:64] = 1.0 / 1.2
    cb[1, 9, 64:128] = 1.0 / 1.2
    cb[0, 10, 0:64] = 1.0
    cb[1, 10, 64:128] = 1.0
    cb[0, 11, :] = 1.0
    epsb = np.full((128, 1), EPS, dtype=np.float32)
    return dict(cb16=cb.astype(BF16), cepsb=epsb)


def _prep_core_inputs(x, ve3, cosb, sinbw, Wq, Wk, Wv, Wo, Wg, consts, b, s):
    g0, g1 = 2 * s, 2 * s + 1
    bf = lambda a: np.ascontiguousarray(a).astype(BF16)
    xt = bf(x[b].T)

    Wq4 = Wq.reshape(HKV, REP, D, C)
    wq_rows = np.concatenate([Wq4[g, r] for r in range(REP) for g in (g0, g1)],
                             axis=0)                       # (512, C)
    wq_full = bf(wq_rows.T)                                # (C, 512)
    Wk3 = Wk.reshape(HKV, D, C)
    wk = bf(np.concatenate([Wk3[g0], Wk3[g1]], axis=0).T)
    Wv3 = Wv.reshape(HKV, D, C)
    wv = bf(np.concatenate([Wv3[g0], Wv3[g1]], axis=0).T)

    Wo4 = Wo.reshape(C, HKV, REP, D)
    wo_cols = np.concatenate([Wo4[:, g, r, :] for r in range(REP)
                              for g in (g0, g1)], axis=1)  # (C, 512)
    wo = bf(wo_cols.T)                                     # (512, C)

    wgn = np.zeros((16, 2), dtype=np.float32)
    wgn[0:GATE_CH, 0] = -Wg[g0]
    wgn[0:GATE_CH, 1] = -Wg[g1]

    ve4 = ve3[b].reshape(T, HKV, D)
    vet2 = np.concatenate([ve4[:, g0, :], ve4[:, g1, :]], axis=1)  # (T, 128)
    vet = bf(vet2.reshape(8, 128, 128).transpose(1, 0, 2))  # (128, 8, 128)

    d = dict(xt=xt, wqa=np.ascontiguousarray(wq_full[:, 0:256]),
             wqb=np.ascontiguousarray(wq_full[:, 256:512]),
             wk=wk, wv=wv, wo=wo, wgn=wgn.astype(BF16),
             vet=vet, cosb=cosb, sinbw=sinbw)
    d.update(consts)
    return d


def kernel(x, ve, cos, sin, Wq, Wk, Wv, Wo, Wg, window_size):
    from concourse.bass_utils import run_bass_kernel_spmd

    assert int(window_size) == WINDOW
    x = np.asarray(x, dtype=np.float32)
    ve = np.asarray(ve, dtype=np.float32)
    Wq = np.asarray(Wq, dtype=np.float32)
    Wk = np.asarray(Wk, dtype=np.float32)
    Wv = np.asarray(Wv, dtype=np.float32)
    Wo = np.asarray(Wo, dtype=np.float32)
    Wg = np.asarray(Wg, dtype=np.float32)
    c = np.asarray(cos, dtype=np.float32).reshape(T, D // 2)   # (T, 32)
    sn = np.asarray(sin, dtype=np.float32).reshape(T, D // 2)

    cosb = np.ascontiguousarray(np.tile(c.T, (4, 1))).astype(BF16)
    sinbw = np.ascontiguousarray(
        np.concatenate([-sn.T, sn.T, -sn.T, sn.T], axis=0)).astype(BF16)
    ve3 = 3.0 * ve
    consts = _const_inputs()

    if "nc" not in _CACHE:
        _CACHE["nc"] = _build_program()
    nc = _CACHE["nc"]

    in_maps = []
    for core in range(NCORES):
        b, s = core // 2, core % 2
        in_maps.append(_prep_core_inputs(x, ve3, cosb, sinbw,
                                         Wq, Wk, Wv, Wo, Wg, consts, b, s))

    res = run_bass_kernel_spmd(nc, in_maps, core_ids=list(range(NCORES)))
    out = np.empty((B, T, C), dtype=np.float32)
    for b in range(B):
        acc = (res.results[2 * b]["out_t"].astype(np.float32)
               + res.results[2 * b + 1]["out_t"].astype(np.float32))
        out[b] = acc.T
    return out


# revision 29
# speedup vs baseline: 1.1611x; 1.0354x over previous
"""Sliding-window GQA causal self-attention for Trainium2, 8 NeuronCores.

Sharding: 8 cores = 4 batches x 2 head-shards. Each core handles one batch
and 2 of the 4 KV groups (8 of 16 Q heads). Core computes a full [C, T]
partial of the output projection in bf16; host sums the two shards per batch.

All matmul operands are bf16 (rel err ~4e-3 total). Band masks are applied
by seeding the score PSUM region with -1e30 via identity matmuls (same
accumulation group as the scores). Softmax denominators come from a ones
column appended to the V stationary. RoPE's partition swap is a permutation
matmul; rsqrt is exp(-0.5*ln(x)) so the Act engine needs one table only.
"""
import numpy as np
import ml_dtypes

B, T, C = 4, 1024, 1024
H, HKV, D = 16, 4, 64
REP = H // HKV
WINDOW = 256
GATE_CH = 12
NCORES = 8
EPS = float(np.finfo(np.float32).eps)
QK_SCALE = 1.2 * 1.2 / 8.0
NEG = -1.0e30
BF16 = ml_dtypes.bfloat16

_CACHE = {}


def _build_program(debug=False, reps=1):
    from contextlib import ExitStack
    import concourse.bass as bass
    import concourse.tile as tile
    from concourse import bacc, mybir

    f32 = mybir.dt.float32
    bf16 = mybir.dt.bfloat16
    ts = bass.ts

    nc = bacc.Bacc("TRN2", target_bir_lowering=False, debug=False,
                   enable_asserts=True, num_devices=NCORES)

    def din(name, shape, dt=bf16):
        return nc.dram_tensor(name, shape, dt, kind="ExternalInput").ap()

    xt = din("xt", [C, T])
    wqa = din("wqa", [C, 256])
    wqb = din("wqb", [C, 256])
    wk = din("wk", [C, 128])
    wv = din("wv", [C, 128])
    wo = din("wo", [512, C])
    wgn = din("wgn", [16, 2])            # negated gate weights (16 = padded)
    vet = din("vet", [128, 8, 128])      # 3*ve, t-major: [t%128, t//128, ch]
    cosb = din("cosb", [128, T])
    sinbw = din("sinbw", [128, T])       # swap32(sin) with sign pattern
    cb16 = din("cb16", [128, 12, 128])   # consts, see _const_inputs
    cepsb = din("cepsb", [128, 1], f32)
    outT = nc.dram_tensor("out_t", [C, T], bf16, kind="ExternalOutput").ap()

    Exp = mybir.ActivationFunctionType.Exp
    Sqrt = mybir.ActivationFunctionType.Sqrt
    Copy = mybir.ActivationFunctionType.Copy
    mult = mybir.AluOpType.mult
    divide = mybir.AluOpType.divide
    add = mybir.AluOpType.add
    bypass = mybir.AluOpType.bypass

    with tile.TileContext(nc) as tc:
     for _rep in range(reps):
      with ExitStack() as ctx:
        sing = ctx.enter_context(tc.tile_pool(name="sing", bufs=1))

        # ---------- persistent tiles + input DMAs (priority order) ----------
        wk_sb = sing.tile([128, 8, 128], bf16, name="wk_sb")
        nc.sync.dma_start(wk_sb[:], wk[:])
        xt_sb = sing.tile([128, 8, T], bf16, name="xt_sb")
        nc.sync.dma_start(xt_sb[:, 0:2, :], xt[0:256, :])
        cb = sing.tile([128, 12, 128], bf16, name="cb")
        nc.sync.dma_start(cb[:], cb16[:])
        wg_sb = sing.tile([16, 2], bf16, name="wg_sb")
        nc.sync.dma_start(wg_sb[:], wgn[:])
        wv_sb = sing.tile([128, 8, 128], bf16, name="wv_sb")
        nc.sync.dma_start(wv_sb[:], wv[:])
        wq_sb = sing.tile([128, 8, 512], bf16, name="wq_sb")
        nc.sync.dma_start(wq_sb[:, :, 0:256], wqa[:])
        nc.sync.dma_start(xt_sb[:, 2:4, :], xt[256:512, :])
        nc.sync.dma_start(xt_sb[:, 4:6, :], xt[512:768, :])
        nc.sync.dma_start(xt_sb[:, 6:8, :], xt[768:1024, :])
        nc.sync.dma_start(wq_sb[:, :, 256:512], wqb[:])
        epsb_sb = sing.tile([128, 1], f32, name="epsb_sb")
        nc.sync.dma_start(epsb_sb[:], cepsb[:])
        cos_sb = sing.tile([128, T], bf16, name="cos_sb")
        nc.sync.dma_start(cos_sb[:], cosb[:])
        sinw_sb = sing.tile([128, T], bf16, name="sinw_sb")
        nc.sync.dma_start(sinw_sb[:], sinbw[:])
        vet_sb = sing.tile([128, 8, 128], bf16, name="vet_sb")
        nc.sync.dma_start(vet_sb[:], vet[:])
        wo_sb = sing.tile([128, 4, C], bf16, name="wo_sb")
        nc.sync.dma_start(wo_sb[:], wo[:])

        ident = cb[:, 0, :]
        pswap = cb[:, 1, :]
        tmask = cb[:, 2:5, :]          # [Tc | 0 | Tw]
        indq4 = [cb[:, 10, 0:4], cb[:, 10, 4:8]]   # even r / odd r
        ind014 = [cb[0:4, 5 + r, :] for r in range(4)]
        indq2 = cb[:, 10, 8:10]

        indbk = cb[0:2, 9, :]
        ones128 = cb[0:1, 11, :]

        kTf = sing.tile([128, T], bf16, name="kTf")
        qTf = [sing.tile([128, T], bf16, name=f"qTf{r}") for r in range(4)]
        v_sb = sing.tile([128, 8, 130], bf16, name="v_sb")
        yTf = [sing.tile([128, T], bf16, name=f"yTf{r}") for r in range(4)]
        nc.vector.memset(v_sb[:, :, 64:65], 1.0)
        nc.vector.memset(v_sb[:, :, 129:130], 1.0)

        # ================= Stage A: projections / rope / rms / gate =========
        with tc.tile_pool(name="stA", bufs=2) as stA, \
             tc.tile_pool(name="pA1", bufs=1, space="PSUM") as pA1:

            k_ps = [pA1.tile([128, 512], f32, name=f"k_ps{h}", tag=f"kps{h}")
                    for h in range(2)]
            v_ps = [pA1.tile([128, 512], f32, name=f"v_ps{h}", tag="vps",
                             bufs=1) for h in range(2)]
            g_ps = [pA1.tile([2, 512], f32, name=f"g_ps{h}", tag="gps",
                             bufs=1) for h in range(2)]

            def tsl(h):
                return slice(512 * h, 512 * h + 512)

            # --- PE: gate projection first (tiny, frees Act/PE early),
            # then k/v chunk-interleaved with the xt DMA arrivals
            for h in range(2):
                nc.tensor.matmul(g_ps[h][:], wg_sb[:], xt_sb[0:16, 0, tsl(h)],
                                 start=True, stop=True)
            for kc in range(8):
                for h in range(2):
                    nc.tensor.matmul(k_ps[h][:], wk_sb[:, kc, :],
                                     xt_sb[:, kc, tsl(h)],
                                     start=(kc == 0), stop=(kc == 7))
                nc.tensor.matmul(v_ps[0][:], wv_sb[:, kc, :],
                                 xt_sb[:, kc, tsl(0)],
                                 start=(kc == 0), stop=(kc == 7))
            for kc in range(8):
                nc.tensor.matmul(v_ps[1][:], wv_sb[:, kc, :],
                                 xt_sb[:, kc, tsl(1)],
                                 start=(kc == 0), stop=(kc == 7))

            # gate chain: expg -> transpose to t-partition cols -> 1/(1+x)
            expg, gT = {}, None
            for h in range(2):
                expg[h] = stA.tile([2, 512], bf16, name="expg", tag="expg",
                                   bufs=2)
                nc.scalar.activation(expg[h][:], g_ps[h][:], Exp)
            gT = pA1.tile([128, 8, 2], bf16, name="gT", tag="sm", bufs=1)
            for h in range(2):
                for tb in range(4):
                    nc.tensor.transpose(gT[:, 4 * h + tb, :],
                                        expg[h][:, ts(tb, 128)],
                                        ident[0:2, 0:2])
            g1sb = stA.tile([128, 8, 2], bf16, name="g1sb", tag="g1", bufs=1)
            nc.vector.scalar_tensor_tensor(g1sb[:], gT[:], 1.0, gT[:],
                                           add, bypass)
            grec = stA.tile([128, 8, 2], bf16, name="grec", tag="grec",
                            bufs=1)
            with nc.allow_low_precision("gate recip bf16"):
                nc.vector.reciprocal(grec[:], g1sb[:])

            # k chain + v copies (Act), rope muls (Pool), squares (DVE)
            k_sb, ku, kc_, k2 = {}, {}, {}, {}
            v_raw = {}
            for h in range(2):
                k_sb[h] = stA.tile([128, 512], bf16, name="k_sb", tag="ksb",
                                   bufs=2)
                nc.scalar.activation(k_sb[h][:], k_ps[h][:], Copy)
                v_raw[h] = stA.tile([128, 512], bf16, name="v_raw", tag="vrw",
                                    bufs=2)
                nc.scalar.activation(v_raw[h][:], v_ps[h][:], Copy)
                ku[h] = sing.tile([128, 512], bf16, name=f"ku{h}")
                nc.gpsimd.tensor_mul(ku[h][:], k_sb[h][:], sinw_sb[:, tsl(h)])
                kc_[h] = sing.tile([128, 512], bf16, name=f"kc{h}")
                nc.gpsimd.tensor_mul(kc_[h][:], k_sb[h][:], cos_sb[:, tsl(h)])
                k2[h] = sing.tile([128, 512], bf16, name=f"k2_{h}")
                nc.vector.tensor_mul(k2[h][:], k_sb[h][:], k_sb[h][:])

            # --- PE: v transposes, then v_sb = gate*ve + v_t (DVE stt)
            vt8 = pA1.tile([128, 8, 128], bf16, name="vt8", tag="vt", bufs=1)
            for h in range(2):
                for tb in range(4):
                    nc.tensor.transpose(vt8[:, 4 * h + tb, :],
                                        v_raw[h][:, ts(tb, 128)], ident[:])
            for jb in range(8):
                for gg in range(2):
                    nc.vector.scalar_tensor_tensor(
                        v_sb[:, jb, 65 * gg:65 * gg + 64],
                        vet_sb[:, jb, 64 * gg:64 * gg + 64],
                        grec[:, jb, gg:gg + 1],
                        vt8[:, jb, 64 * gg:64 * gg + 64], mult, add)

            # (k rope swap happens in pool 2)
            # --- q projections + psum->sbuf copy + rope muls + squares
            # (allocated in pool 1 so they overlap the xt DMA phase)
            q_ps, q_sbs = {}, {}
            for r in range(4):
                for h in range(2):
                    qp = pA1.tile([128, 512], f32, name=f"q_ps{r}{h}",
                                  tag="qps", bufs=2)
                    for kc in range(8):
                        nc.tensor.matmul(qp[:], wq_sb[:, kc, ts(r, 128)],
                                         xt_sb[:, kc, tsl(h)],
                                         start=(kc == 0), stop=(kc == 7))
                    q_ps[(r, h)] = qp
                    qs = sing.tile([128, 512], bf16, name=f"q_sb{r}{h}")
                    nc.scalar.activation(qs[:], qp[:], Copy)
                    q_sbs[(r, h)] = qs

        with tc.tile_pool(name="stA2", bufs=2) as stA, \
             tc.tile_pool(name="pA2", bufs=1, space="PSUM") as pA2:
            def tsl(h):
                return slice(512 * h, 512 * h + 512)

            # --- k rope swap + rms-fold (divide), k-ms
            ksw_ps, kpre = {}, {}
            for h in range(2):
                ksw_ps[h] = pA2.tile([128, 512], f32, name="ksw_ps",
                                     tag="sw", bufs=2)
                nc.tensor.matmul(ksw_ps[h][:], pswap[:], ku[h][:],
                                 start=True, stop=True)
                kpre[h] = stA.tile([128, 512], bf16, name=f"kpre{h}",
                                   tag="kpre", bufs=2)
                nc.vector.tensor_add(kpre[h][:], ksw_ps[h][:], kc_[h][:])
            msk_ps, sqk = {}, {}
            for h in range(2):
                msk_ps[h] = pA2.tile([2, 512], f32, name="msk_ps", tag="msk",
                                     bufs=1)
                nc.tensor.matmul(msk_ps[h][:], indq2[:], k2[h][:],
                                 start=True, stop=True)
                sqk[h] = stA.tile([2, 512], bf16, name="sqk", tag="sqk",
                                  bufs=2)
                nc.scalar.activation(sqk[h][:], msk_ps[h][:], Sqrt,
                                     bias=epsb_sb[0:2, :])
            for h in range(2):
                rkb_ps = pA2.tile([128, 512], f32, name="rkb_ps",
                                  tag="bc", bufs=2)
                nc.tensor.matmul(rkb_ps[:], indbk[:], sqk[h][:],
                                 start=True, stop=True)
                nc.vector.tensor_tensor(kTf[:, tsl(h)], kpre[h][:],
                                        rkb_ps[:], divide)

            # --- q rope + rms, r-major, divide-fold
            msq_ps = {}

            for r in range(4):
                qu, qc2, q2s, qsw = {}, {}, {}, {}
                for h in range(2):
                    qs = q_sbs[(r, h)]
                    u = stA.tile([128, 512], bf16, name="qu", tag="qu",
                                 bufs=2)
                    nc.gpsimd.tensor_mul(u[:], qs[:], sinw_sb[:, tsl(h)])
                    qu[h] = u
                    c2 = stA.tile([128, 512], bf16, name="qc_", tag="qc",
                                  bufs=2)
                    nc.gpsimd.tensor_mul(c2[:], qs[:], cos_sb[:, tsl(h)])
                    qc2[h] = c2
                    s2 = stA.tile([128, 512], bf16, name="q2", tag="q2",
                                  bufs=2)
                    nc.vector.tensor_mul(s2[:], qs[:], qs[:])
                    q2s[h] = s2
                for h in range(2):
                    sw = pA2.tile([128, 512], f32, name="qsw_ps", tag="sw",
                                  bufs=2)
                    nc.tensor.matmul(sw[:], pswap[:], qu[h][:],
                                     start=True, stop=True)
                    qsw[h] = sw
                    if r % 2 == 0:
                        msq_ps[(h, r // 2)] = pA2.tile(
                            [4, 512], f32, name=f"msq{h}", tag=f"msq{h}",
                            bufs=1)
                    nc.tensor.matmul(msq_ps[(h, r // 2)][:], indq4[r % 2][:],
                                     q2s[h][:], start=(r % 2 == 0),
                                     stop=(r % 2 == 1), skip_group_check=True)
                qpre = {}
                for h in range(2):
                    qp = stA.tile([128, 512], bf16, name="qpre", tag="qpre",
                                  bufs=4)
                    nc.vector.tensor_add(qp[:], qsw[h][:], qc2[h][:])
                    qpre[(r, h)] = qp
                globals().setdefault("_qpre_all", {}).update(qpre)
                if r % 2 == 1:
                    p = r // 2
                    for h in range(2):
                        sq4 = stA.tile([4, 512], bf16, name="sq4", tag="sq4",
                                       bufs=2)
                        nc.scalar.activation(sq4[:], msq_ps[(h, p)][:], Sqrt,
                                             bias=epsb_sb[0:4, :])
                        for rr in (2 * p, 2 * p + 1):
                            rb_ps = pA2.tile([128, 512], f32, name="rb_ps",
                                             tag="bc", bufs=2)
                            nc.tensor.matmul(rb_ps[:], ind014[rr][:], sq4[:],
                                             start=True, stop=True)
                            nc.vector.tensor_tensor(
                                qTf[rr][:, tsl(h)],
                                globals()["_qpre_all"][(rr, h)][:],
                                rb_ps[:], divide)

        # ================= Stage B: attention ================================
        with tc.tile_pool(name="stB", bufs=2) as stB, \
             tc.tile_pool(name="pB_", bufs=1, space="PSUM") as pB_:
            for r in range(4):
                pkeep = {j: stB.tile([128, 2, 3, 128], bf16, name=f"pk{j}",
                                     tag=f"pk{j}", bufs=2) for j in (2, 3)}
                for h in range(2):
                    hsl = slice(512 * h, 512 * h + 512)
                    y_ps = pB_.tile([65, 2, 512], f32, name="y_ps",
                                    tag="yps", bufs=2)
                    jlist = list(range(0, 4)) if h == 0 else list(range(2, 8))
                    first = True
                    for j in jlist:
                        w = min(384, T - 128 * j)
                        ns = w // 128          # number of 128-wide segments
                        fresh = not (h == 1 and j in (2, 3))
                        if not fresh:
                            p2 = pkeep[j]      # cached from h == 0
                        else:
                            p2 = pkeep.get(j)
                            if p2 is None:
                                p2 = stB.tile([128, 2, 3, 128], bf16,
                                              name="p2", tag="p2", bufs=3)
                            sc2 = pB_.tile([128, 2, 3, 128], f32, name="sc2",
                                           tag="sc", bufs=2)
                            for gg in range(2):
                                dsl = slice(64 * gg, 64 * gg + 64)
                                qsl = slice(128 * j, 128 * j + w)
                                nc.tensor.matmul(
                                    sc2[:, gg, 0:ns, :], ident,
                                    tmask[:, 0:ns, :], start=True, stop=False,
                                    skip_group_check=True)
                                nc.tensor.matmul(
                                    sc2[:, gg, 0:ns, :],
                                    kTf[dsl, 128 * j:128 * j + 128],
                                    qTf[r][dsl, qsl], start=False, stop=True,
                                    skip_group_check=True)
                            nc.scalar.activation(p2[:, :, 0:ns, :],
                                                 sc2[:, :, 0:ns, :], Exp)
                        a = max(128 * j, 512 * h)
                        b = min(128 * j + w, 512 * h + 512)
                        s0, s1 = (a - 128 * j) // 128, (b - 128 * j) // 128
                        for gg in range(2):
                            nc.tensor.matmul(
                                y_ps[:, gg, a - 512 * h:b - 512 * h],
                                v_sb[:, j, 65 * gg:65 * gg + 65],
                                p2[:, gg, s0:s1, :],
                                start=first, stop=(j == jlist[-1]),
                                skip_group_check=True)
                        first = False
                    # normalize: 1/sums, broadcast via ones matmul, 2 muls
                    rsum = stB.tile([1, 2, 512], bf16, name="rsum",
                                    tag="rsum", bufs=2)
                    with nc.allow_low_precision("1/sums bf16"):
                        nc.vector.reciprocal(rsum[:], y_ps[64:65, :, :])
                    rbs_ps = pB_.tile([128, 512], f32, name="rbs_ps",
                                      tag="sc", bufs=2)
                    nc.tensor.matmul(rbs_ps[0:64, :], ones128[:, 0:64],
                                     rsum[:, 0, :], start=True, stop=True)
                    nc.tensor.matmul(rbs_ps[64:128, :], ones128[:, 0:64],
                                     rsum[:, 1, :], start=True, stop=True,
                                     skip_group_check=True)
                    rbs_sb = stB.tile([128, 512], bf16, name="rbs_sb",
                                      tag="rbs_sb", bufs=2)
                    nc.scalar.activation(rbs_sb[:], rbs_ps[:], Copy)
                    for gg in range(2):
                        nc.vector.tensor_mul(yTf[r][ts(gg, 64), hsl],
                                             y_ps[0:64, gg, :],
                                             rbs_sb[ts(gg, 64), :])

            # ---- Stage C: output projection (same pool: overlaps attention)
            for h in range(2):
                hsl = slice(512 * h, 512 * h + 512)
                for ct in range(8):
                    o_ps = pB_.tile([128, 512], f32, name="o_ps", tag="yps",
                                    bufs=2)
                    for kr in range(4):
                        nc.tensor.matmul(o_ps[:], wo_sb[:, kr, ts(ct, 128)],
                                         yTf[kr][:, hsl], start=(kr == 0),
                                         stop=(kr == 3))
                    o_sb = stB.tile([128, 512], bf16, name="o_sb", tag="osb",
                                    bufs=3)
                    if ct % 2 == 0:
                        nc.vector.tensor_copy(o_sb[:], o_ps[:])
                    else:
                        nc.scalar.activation(o_sb[:], o_ps[:], Copy)
                    nc.sync.dma_start(outT[ts(ct, 128), hsl], o_sb[:])

    nc.compile()
    return nc


def _const_inputs():
    cb = np.zeros((128, 12, 128), dtype=np.float32)
    # 0: identity
    cb[:, 0, :] = np.eye(128, dtype=np.float32)
    # 1: pswap  P[c, m] = 1 iff c == swap(m), swap = +-32 within 64-block
    m = np.arange(128)
    sw = np.where((m % 64) < 32, m + 32, m - 32)
    cb[sw, 1, m] = 1.0
    # 2: Tc diag mask, 3: zeros (mid), 4: Tw window mask -> contiguous seed
    p = np.arange(128)[:, None]
    c = np.arange(128)[None, :]
    cb[:, 2, :] = np.where(c >= p, 0.0, NEG)
    cb[:, 4, :] = np.where(c <= p, 0.0, NEG)
    # 10: cols 0:4 = msq stationary for even r, 4:8 odd r, 8:10 indq2 (k)
    cb[0:64, 10, 0] = 1.0 / D
    cb[64:128, 10, 1] = 1.0 / D
    cb[0:64, 10, 6] = 1.0 / D
    cb[64:128, 10, 7] = 1.0 / D
    cb[0:64, 10, 8] = 1.0 / D
    cb[64:128, 10, 9] = 1.0 / D

    # 5..8: ind014 per r: rsq4 row (2*(r%2)+gg) -> out gg rows, val 1/QK_SCALE
    for r in range(4):
        i = r % 2
        cb[2 * i, 5 + r, 0:64] = 1.0 / QK_SCALE
        cb[2 * i + 1, 5 + r, 64:128] = 1.0 / QK_SCALE
    # 9: indbk rows 0:2 (1/1.2), 10: indbg rows 0:2 (1.0), 11: ones row 0
    cb[0, 9, 0:64] = 1.0 / 1.2
    cb[1, 9, 64:128] = 1.0 / 1.2
    cb[0, 11, :] = 1.0
    epsb = np.full((128, 1), EPS, dtype=np.float32)
    return dict(cb16=cb.astype(BF16), cepsb=epsb)


def _prep_core_inputs(x, ve3, cosb, sinbw, Wq, Wk, Wv, Wo, Wg, consts, b, s):
    g0, g1 = 2 * s, 2 * s + 1
    bf = lambda a: np.ascontiguousarray(a).astype(BF16)
    xt = bf(x[b].T)

    Wq4 = Wq.reshape(HKV, REP, D, C)
    wq_rows = np.concatenate([Wq4[g, r] for r in range(REP) for g in (g0, g1)],
                             axis=0)                       # (512, C)
    wq_full = bf(wq_rows.T)                                # (C, 512)
    Wk3 = Wk.reshape(HKV, D, C)
    wk = bf(np.concatenate([Wk3[g0], Wk3[g1]], axis=0).T)
    Wv3 = Wv.reshape(HKV, D, C)
    wv = bf(np.concatenate([Wv3[g0], Wv3[g1]], axis=0).T)

    Wo4 = Wo.reshape(C, HKV, REP, D)
    wo_cols = np.concatenate([Wo4[:, g, r, :] for r in range(REP)
                              for g in (g0, g1)], axis=1)  # (C, 512)
    wo = bf(wo_cols.T)                                     # (512, C)

    wgn = np.zeros((16, 2), dtype=np.float32)
    wgn[0:GATE_CH, 0] = -Wg[g0]
    wgn[0:GATE_CH, 1] = -Wg[g1]

    ve4 = ve3[b].reshape(T, HKV, D)
    vet2 = np.concatenate([ve4[:, g0, :], ve4[:, g1, :]], axis=1)  # (T, 128)
    vet = bf(vet2.reshape(8, 128, 128).transpose(1, 0, 2))  # (128, 8, 128)

    d = dict(xt=xt, wqa=np.ascontiguousarray(wq_full[:, 0:256]),
             wqb=np.ascontiguousarray(wq_full[:, 256:512]),
             wk=wk, wv=wv, wo=wo, wgn=wgn.astype(BF16),
             vet=vet, cosb=cosb, sinbw=sinbw)
    d.update(consts)
    return d


def kernel(x, ve, cos, sin, Wq, Wk, Wv, Wo, Wg, window_size):
    from concourse.bass_utils import run_bass_kernel_spmd

    assert int(window_size) == WINDOW
    x = np.asarray(x, dtype=np.float32)
    ve = np.asarray(ve, dtype=np.float32)
    Wq = np.asarray(Wq, dtype=np.float32)
    Wk = np.asarray(Wk, dtype=np.float32)
    Wv = np.asarray(Wv, dtype=np.float32)
    Wo = np.asarray(Wo, dtype=np.float32)
    Wg = np.asarray(Wg, dtype=np.float32)
    c = np.asarray(cos, dtype=np.float32).reshape(T, D // 2)   # (T, 32)
    sn = np.asarray(sin, dtype=np.float32).reshape(T, D // 2)

    cosb = np.ascontiguousarray(np.tile(c.T, (4, 1))).astype(BF16)
    sinbw = np.ascontiguousarray(
        np.concatenate([-sn.T, sn.T, -sn.T, sn.T], axis=0)).astype(BF16)
    ve3 = 3.0 * ve
    consts = _const_inputs()

    if "nc" not in _CACHE:
        _CACHE["nc"] = _build_program()
    nc = _CACHE["nc"]

    in_maps = []
    for core in range(NCORES):
        b, s = core // 2, core % 2
        in_maps.append(_prep_core_inputs(x, ve3, cosb, sinbw,
                                         Wq, Wk, Wv, Wo, Wg, consts, b, s))

    res = run_bass_kernel_spmd(nc, in_maps, core_ids=list(range(NCORES)))
    out = np.empty((B, T, C), dtype=np.float32)
    for b in range(B):
        acc = (res.results[2 * b]["out_t"].astype(np.float32)
               + res.results[2 * b + 1]["out_t"].astype(np.float32))
        out[b] = acc.T
    return out


# revision 30
# speedup vs baseline: 1.3144x; 1.1320x over previous
"""Sliding-window GQA causal self-attention for Trainium2, 8 NeuronCores.

Sharding: 8 cores = 4 batches x 2 head-shards. Each core handles one batch
and 2 of the 4 KV groups (8 of 16 Q heads). Core computes a full [C, T]
partial of the output projection in bf16; host sums the two shards per batch.

All matmul operands are bf16 (rel err ~4e-3 total). Band masks are applied
by seeding the score PSUM region with -1e30 via identity matmuls (same
accumulation group as the scores). Softmax denominators come from a ones
column appended to the V stationary. RoPE's partition swap is a permutation
matmul; rsqrt is exp(-0.5*ln(x)) so the Act engine needs one table only.
"""
import numpy as np
import ml_dtypes

B, T, C = 4, 1024, 1024
H, HKV, D = 16, 4, 64
REP = H // HKV
WINDOW = 256
GATE_CH = 12
NCORES = 8
EPS = float(np.finfo(np.float32).eps)
QK_SCALE = 1.2 * 1.2 / 8.0
NEG = -1.0e30
BF16 = ml_dtypes.bfloat16

_CACHE = {}


def _build_program(debug=False, reps=1):
    from contextlib import ExitStack
    import concourse.bass as bass
    import concourse.tile as tile
    from concourse import bacc, mybir

    f32 = mybir.dt.float32
    bf16 = mybir.dt.bfloat16
    ts = bass.ts

    nc = bacc.Bacc("TRN2", target_bir_lowering=False, debug=False,
                   enable_asserts=True, num_devices=NCORES)

    def din(name, shape, dt=bf16):
        return nc.dram_tensor(name, shape, dt, kind="ExternalInput").ap()

    xt = din("xt", [C, T])
    wqa = din("wqa", [C, 256])
    wqb = din("wqb", [C, 256])
    wk = din("wk", [C, 128])
    wv = din("wv", [C, 128])
    wo = din("wo", [512, C])
    wgn = din("wgn", [16, 2])            # negated gate weights (16 = padded)
    vet = din("vet", [128, 8, 128])      # 3*ve, t-major: [t%128, t//128, ch]
    cosb = din("cosb", [128, T])
    sinbw = din("sinbw", [128, T])       # swap32(sin) with sign pattern
    cb16 = din("cb16", [128, 12, 128])   # consts, see _const_inputs
    cepsb = din("cepsb", [128, 1], f32)
    outT = nc.dram_tensor("out_t", [C, T], bf16, kind="ExternalOutput").ap()

    Exp = mybir.ActivationFunctionType.Exp
    Sqrt = mybir.ActivationFunctionType.Sqrt
    Copy = mybir.ActivationFunctionType.Copy
    mult = mybir.AluOpType.mult
    divide = mybir.AluOpType.divide
    add = mybir.AluOpType.add
    bypass = mybir.AluOpType.bypass

    with tile.TileContext(nc) as tc:
     for _rep in range(reps):
      with ExitStack() as ctx:
        sing = ctx.enter_context(tc.tile_pool(name="sing", bufs=1))

        # ---------- persistent tiles + input DMAs (priority order) ----------
        wk_sb = sing.tile([128, 8, 128], bf16, name="wk_sb")
        nc.sync.dma_start(wk_sb[:], wk[:])
        xt_sb = sing.tile([128, 8, T], bf16, name="xt_sb")
        nc.sync.dma_start(xt_sb[:, 0:2, :], xt[0:256, :])
        cb = sing.tile([128, 12, 128], bf16, name="cb")
        nc.sync.dma_start(cb[:], cb16[:])
        wg_sb = sing.tile([16, 2], bf16, name="wg_sb")
        nc.sync.dma_start(wg_sb[:], wgn[:])
        wv_sb = sing.tile([128, 8, 128], bf16, name="wv_sb")
        nc.sync.dma_start(wv_sb[:], wv[:])
        wq_sb = sing.tile([128, 8, 512], bf16, name="wq_sb")
        nc.sync.dma_start(wq_sb[:, :, 0:256], wqa[:])
        nc.sync.dma_start(xt_sb[:, 2:4, :], xt[256:512, :])
        nc.sync.dma_start(xt_sb[:, 4:6, :], xt[512:768, :])
        nc.sync.dma_start(xt_sb[:, 6:8, :], xt[768:1024, :])
        nc.sync.dma_start(wq_sb[:, :, 256:512], wqb[:])
        epsb_sb = sing.tile([128, 1], f32, name="epsb_sb")
        nc.sync.dma_start(epsb_sb[:], cepsb[:])
        cos_sb = sing.tile([128, T], bf16, name="cos_sb")
        nc.sync.dma_start(cos_sb[:], cosb[:])
        sinw_sb = sing.tile([128, T], bf16, name="sinw_sb")
        nc.sync.dma_start(sinw_sb[:], sinbw[:])
        vet_sb = sing.tile([128, 8, 128], bf16, name="vet_sb")
        nc.sync.dma_start(vet_sb[:], vet[:])
        wo_sb = sing.tile([128, 4, C], bf16, name="wo_sb")
        nc.sync.dma_start(wo_sb[:], wo[:])

        ident = cb[:, 0, :]
        pswap = cb[:, 1, :]
        tmask = cb[:, 2:5, :]          # [Tc | 0 | Tw]
        indq4 = [cb[:, 10, 0:4], cb[:, 10, 4:8]]   # even r / odd r
        ind014 = [cb[0:4, 5 + r, :] for r in range(4)]
        indq2 = cb[:, 10, 8:10]

        indbk = cb[0:2, 9, :]
        ones128 = cb[0:1, 11, :]

        kTf = sing.tile([128, T], bf16, name="kTf")
        qTf = [sing.tile([128, T], bf16, name=f"qTf{r}") for r in range(4)]
        v_sb = sing.tile([128, 8, 130], bf16, name="v_sb")
        yTf = [sing.tile([128, T], bf16, name=f"yTf{r}") for r in range(4)]
        nc.vector.memset(v_sb[:, :, 64:65], 1.0)
        nc.vector.memset(v_sb[:, :, 129:130], 1.0)

        # ================= Stage A: projections / rope / rms / gate =========
        with tc.tile_pool(name="stA", bufs=2) as stA, \
             tc.tile_pool(name="pA1", bufs=1, space="PSUM") as pA1:

            k_ps = [pA1.tile([128, 512], f32, name=f"k_ps{h}", tag=f"kps{h}")
                    for h in range(2)]
            v_ps = [pA1.tile([128, 512], f32, name=f"v_ps{h}", tag="vps",
                             bufs=1) for h in range(2)]
            g_ps = [pA1.tile([2, 512], f32, name=f"g_ps{h}", tag="gps",
                             bufs=1) for h in range(2)]

            def tsl(h):
                return slice(512 * h, 512 * h + 512)

            # --- PE: gate projection first (tiny, frees Act/PE early),
            # then k/v chunk-interleaved with the xt DMA arrivals
            for h in range(2):
                nc.tensor.matmul(g_ps[h][:], wg_sb[:], xt_sb[0:16, 0, tsl(h)],
                                 start=True, stop=True)
            for kc in range(8):
                for h in range(2):
                    nc.tensor.matmul(k_ps[h][:], wk_sb[:, kc, :],
                                     xt_sb[:, kc, tsl(h)],
                                     start=(kc == 0), stop=(kc == 7))
                nc.tensor.matmul(v_ps[0][:], wv_sb[:, kc, :],
                                 xt_sb[:, kc, tsl(0)],
                                 start=(kc == 0), stop=(kc == 7))
            for kc in range(8):
                nc.tensor.matmul(v_ps[1][:], wv_sb[:, kc, :],
                                 xt_sb[:, kc, tsl(1)],
                                 start=(kc == 0), stop=(kc == 7))

            # gate chain: expg -> transpose to t-partition cols -> 1/(1+x)
            expg, gT = {}, None
            for h in range(2):
                expg[h] = stA.tile([2, 512], bf16, name="expg", tag="expg",
                                   bufs=2)
                nc.scalar.activation(expg[h][:], g_ps[h][:], Exp)
            gT = pA1.tile([128, 8, 2], bf16, name="gT", tag="sm", bufs=1)
            for h in range(2):
                for tb in range(4):
                    nc.tensor.transpose(gT[:, 4 * h + tb, :],
                                        expg[h][:, ts(tb, 128)],
                                        ident[0:2, 0:2])
            g1sb = stA.tile([128, 8, 2], bf16, name="g1sb", tag="g1", bufs=1)
            nc.vector.scalar_tensor_tensor(g1sb[:], gT[:], 1.0, gT[:],
                                           add, bypass)
            grec = stA.tile([128, 8, 2], bf16, name="grec", tag="grec",
                            bufs=1)
            with nc.allow_low_precision("gate recip bf16"):
                nc.vector.reciprocal(grec[:], g1sb[:])

            # k chain + v copies (Act), rope muls (Pool), squares (DVE)
            k_sb, ku, kc_, k2 = {}, {}, {}, {}
            v_raw = {}
            for h in range(2):
                k_sb[h] = stA.tile([128, 512], bf16, name="k_sb", tag="ksb",
                                   bufs=2)
                nc.scalar.activation(k_sb[h][:], k_ps[h][:], Copy)
                v_raw[h] = stA.tile([128, 512], bf16, name="v_raw", tag="vrw",
                                    bufs=2)
                nc.scalar.activation(v_raw[h][:], v_ps[h][:], Copy)
                ku[h] = sing.tile([128, 512], bf16, name=f"ku{h}")
                nc.gpsimd.tensor_mul(ku[h][:], k_sb[h][:], sinw_sb[:, tsl(h)])
                kc_[h] = sing.tile([128, 512], bf16, name=f"kc{h}")
                nc.gpsimd.tensor_mul(kc_[h][:], k_sb[h][:], cos_sb[:, tsl(h)])
                k2[h] = sing.tile([128, 512], bf16, name=f"k2_{h}")
                nc.vector.tensor_mul(k2[h][:], k_sb[h][:], k_sb[h][:])

            # --- PE: v transposes, then v_sb = gate*ve + v_t (DVE stt)
            vt8 = pA1.tile([128, 8, 128], bf16, name="vt8", tag="vt", bufs=1)
            for h in range(2):
                for tb in range(4):
                    nc.tensor.transpose(vt8[:, 4 * h + tb, :],
                                        v_raw[h][:, ts(tb, 128)], ident[:])
            for jb in range(8):
                for gg in range(2):
                    nc.vector.scalar_tensor_tensor(
                        v_sb[:, jb, 65 * gg:65 * gg + 64],
                        vet_sb[:, jb, 64 * gg:64 * gg + 64],
                        grec[:, jb, gg:gg + 1],
                        vt8[:, jb, 64 * gg:64 * gg + 64], mult, add)

            # (k rope swap happens in pool 2)
            # --- q projections + psum->sbuf copy + rope muls + squares
            # (allocated in pool 1 so they overlap the xt DMA phase)
            q_ps, q_sbs = {}, {}
            for r in range(4):
                for h in range(2):
                    qp = pA1.tile([128, 512], f32, name=f"q_ps{r}{h}",
                                  tag="qps", bufs=2)
                    for kc in range(8):
                        nc.tensor.matmul(qp[:], wq_sb[:, kc, ts(r, 128)],
                                         xt_sb[:, kc, tsl(h)],
                                         start=(kc == 0), stop=(kc == 7))
                    q_ps[(r, h)] = qp
                    qs = sing.tile([128, 512], bf16, name=f"q_sb{r}{h}")
                    nc.scalar.activation(qs[:], qp[:], Copy)
                    q_sbs[(r, h)] = qs

        with tc.tile_pool(name="stA2", bufs=2) as stA, \
             tc.tile_pool(name="pA2", bufs=1, space="PSUM") as pA2:
            def tsl(h):
                return slice(512 * h, 512 * h + 512)

            # --- k rope swap + rms-fold (divide), k-ms
            ksw_ps, kpre = {}, {}
            for h in range(2):
                ksw_ps[h] = pA2.tile([128, 512], f32, name="ksw_ps",
                                     tag="sw", bufs=3)
                nc.tensor.matmul(ksw_ps[h][:], pswap[:], ku[h][:],
                                 start=True, stop=True)
                kpre[h] = stA.tile([128, 512], bf16, name=f"kpre{h}",
                                   tag="kpre", bufs=2)
                nc.vector.tensor_add(kpre[h][:], ksw_ps[h][:], kc_[h][:])
            msk_ps, sqk = {}, {}
            for h in range(2):
                msk_ps[h] = pA2.tile([2, 512], f32, name="msk_ps", tag="msk",
                                     bufs=1)
                nc.tensor.matmul(msk_ps[h][:], indq2[:], k2[h][:],
                                 start=True, stop=True)
                sqk[h] = stA.tile([2, 512], bf16, name="sqk", tag="sqk",
                                  bufs=2)
                nc.scalar.activation(sqk[h][:], msk_ps[h][:], Sqrt,
                                     bias=epsb_sb[0:2, :])
            for h in range(2):
                rkb_ps = pA2.tile([128, 512], f32, name="rkb_ps",
                                  tag="bc", bufs=2)
                nc.tensor.matmul(rkb_ps[:], indbk[:], sqk[h][:],
                                 start=True, stop=True)
                nc.vector.tensor_tensor(kTf[:, tsl(h)], kpre[h][:],
                                        rkb_ps[:], divide)

            # --- q rope + rms, r-major, divide-fold
            msq_ps = {}

            for r in range(4):
                qu, qc2, q2s, qsw = {}, {}, {}, {}
                for h in range(2):
                    qs = q_sbs[(r, h)]
                    u = stA.tile([128, 512], bf16, name="qu", tag="qu",
                                 bufs=2)
                    nc.gpsimd.tensor_mul(u[:], qs[:], sinw_sb[:, tsl(h)])
                    qu[h] = u
                    c2 = stA.tile([128, 512], bf16, name="qc_", tag="qc",
                                  bufs=2)
                    nc.gpsimd.tensor_mul(c2[:], qs[:], cos_sb[:, tsl(h)])
                    qc2[h] = c2
                    s2 = stA.tile([128, 512], bf16, name="q2", tag="q2",
                                  bufs=2)
                    nc.vector.tensor_mul(s2[:], qs[:], qs[:])
                    q2s[h] = s2
                for h in range(2):
                    sw = pA2.tile([128, 512], f32, name="qsw_ps", tag="sw",
                                  bufs=3)
                    nc.tensor.matmul(sw[:], pswap[:], qu[h][:],
                                     start=True, stop=True)
                    qsw[h] = sw
                    if r % 2 == 0:
                        msq_ps[(h, r // 2)] = pA2.tile(
                            [4, 512], f32, name=f"msq{h}", tag=f"msq{h}",
                            bufs=1)
                    nc.tensor.matmul(msq_ps[(h, r // 2)][:], indq4[r % 2][:],
                                     q2s[h][:], start=(r % 2 == 0),
                                     stop=(r % 2 == 1), skip_group_check=True)
                qpre = {}
                for h in range(2):
                    qp = stA.tile([128, 512], bf16, name="qpre", tag="qpre",
                                  bufs=4)
                    nc.vector.tensor_add(qp[:], qsw[h][:], qc2[h][:])
                    qpre[(r, h)] = qp
                globals().setdefault("_qpre_all", {}).update(qpre)
                if r % 2 == 1:
                    p = r // 2
                    for h in range(2):
                        sq4 = stA.tile([4, 512], bf16, name="sq4", tag="sq4",
                                       bufs=2)
                        nc.scalar.activation(sq4[:], msq_ps[(h, p)][:], Sqrt,
                                             bias=epsb_sb[0:4, :])
                        for rr in (2 * p, 2 * p + 1):
                            rb_ps = pA2.tile([128, 512], f32, name="rb_ps",
                                             tag="bc", bufs=2)
                            nc.tensor.matmul(rb_ps[:], ind014[rr][:], sq4[:],
                                             start=True, stop=True)
                            nc.vector.tensor_tensor(
                                qTf[rr][:, tsl(h)],
                                globals()["_qpre_all"][(rr, h)][:],
                                rb_ps[:], divide)

        # ================= Stage B: attention ================================
        with tc.tile_pool(name="stB", bufs=2) as stB, \
             tc.tile_pool(name="pB_", bufs=1, space="PSUM") as pB_:
            pending_norm = []

            def flush_norm():
                while pending_norm:
                    pending_norm.pop(0)()

            for r in range(4):
                pkeep = {j: stB.tile([128, 2, 3, 128], bf16, name=f"pk{j}",
                                     tag=f"pk{j}", bufs=2) for j in (2, 3)}
                for h in range(2):
                    hsl = slice(512 * h, 512 * h + 512)
                    y_ps = pB_.tile([65, 2, 512], f32, name="y_ps",
                                    tag="yps", bufs=2)
                    jlist = list(range(0, 4)) if h == 0 else list(range(2, 8))
                    first = True
                    for j in jlist:
                        w = min(384, T - 128 * j)
                        ns = w // 128          # number of 128-wide segments
                        fresh = not (h == 1 and j in (2, 3))
                        if not fresh:
                            p2 = pkeep[j]      # cached from h == 0
                        else:
                            p2 = pkeep.get(j)
                            if p2 is None:
                                p2 = stB.tile([128, 2, 3, 128], bf16,
                                              name="p2", tag="p2", bufs=3)
                            sc2 = pB_.tile([128, 2, 3, 128], f32, name="sc2",
                                           tag="sc", bufs=2)
                            for gg in range(2):
                                dsl = slice(64 * gg, 64 * gg + 64)
                                qsl = slice(128 * j, 128 * j + w)
                                nc.tensor.matmul(
                                    sc2[:, gg, 0:ns, :], ident,
                                    tmask[:, 0:ns, :], start=True, stop=False,
                                    skip_group_check=True)
                                nc.tensor.matmul(
                                    sc2[:, gg, 0:ns, :],
                                    kTf[dsl, 128 * j:128 * j + 128],
                                    qTf[r][dsl, qsl], start=False, stop=True,
                                    skip_group_check=True)
                            nc.scalar.activation(p2[:, :, 0:ns, :],
                                                 sc2[:, :, 0:ns, :], Exp)
                            if j == jlist[0]:
                                flush_norm()
                        a = max(128 * j, 512 * h)
                        b = min(128 * j + w, 512 * h + 512)
                        s0, s1 = (a - 128 * j) // 128, (b - 128 * j) // 128
                        for gg in range(2):
                            nc.tensor.matmul(
                                y_ps[:, gg, a - 512 * h:b - 512 * h],
                                v_sb[:, j, 65 * gg:65 * gg + 65],
                                p2[:, gg, s0:s1, :],
                                start=first, stop=(j == jlist[-1]),
                                skip_group_check=True)
                        first = False

                    def make_norm(r=r, h=h, hsl=hsl, y_ps=y_ps):
                        def emit():
                            rsum = stB.tile([1, 2, 512], bf16, name="rsum",
                                            tag="rsum", bufs=2)
                            with nc.allow_low_precision("1/sums bf16"):
                                nc.vector.reciprocal(rsum[:],
                                                     y_ps[64:65, :, :])
                            rbs_ps = pB_.tile([128, 512], f32, name="rbs_ps",
                                              tag="sc", bufs=2)
                            nc.tensor.matmul(rbs_ps[0:64, :],
                                             ones128[:, 0:64], rsum[:, 0, :],
                                             start=True, stop=True)
                            nc.tensor.matmul(rbs_ps[64:128, :],
                                             ones128[:, 0:64], rsum[:, 1, :],
                                             start=True, stop=True,
                                             skip_group_check=True)
                            rbs_sb = stB.tile([128, 512], bf16, name="rbs_sb",
                                              tag="rbs_sb", bufs=2)
                            nc.scalar.activation(rbs_sb[:], rbs_ps[:], Copy)
                            for gg in range(2):
                                nc.vector.tensor_mul(yTf[r][ts(gg, 64), hsl],
                                                     y_ps[0:64, gg, :],
                                                     rbs_sb[ts(gg, 64), :])
                        return emit
                    pending_norm.append(make_norm())
            flush_norm()

            # ---- Stage C: output projection (same pool: overlaps attention)
            for h in range(2):
                hsl = slice(512 * h, 512 * h + 512)
                for cp in range(4):
                    o_ps = pB_.tile([128, 2, 512], f32, name="o_ps", tag="sc",
                                    bufs=2)
                    for ci in range(2):
                        ct = 2 * cp + ci
                        for kr in range(4):
                            nc.tensor.matmul(o_ps[:, ci, :],
                                             wo_sb[:, kr, ts(ct, 128)],
                                             yTf[kr][:, hsl],
                                             start=(kr == 0), stop=(kr == 3),
                                             skip_group_check=True)
                    o_sb = stB.tile([128, 2, 512], bf16, name="o_sb",
                                    tag="osb", bufs=3)
                    if cp % 2 == 0:
                        nc.vector.tensor_copy(o_sb[:], o_ps[:, :, :])
                    else:
                        nc.scalar.activation(o_sb[:], o_ps[:, :, :], Copy)
                    nc.sync.dma_start(
                        outT[256 * cp:256 * cp + 256, hsl].rearrange(
                            "(c p) t -> p c t", c=2), o_sb[:])

    nc.compile()
    return nc


def _const_inputs():
    cb = np.zeros((128, 12, 128), dtype=np.float32)
    # 0: identity
    cb[:, 0, :] = np.eye(128, dtype=np.float32)
    # 1: pswap  P[c, m] = 1 iff c == swap(m), swap = +-32 within 64-block
    m = np.arange(128)
    sw = np.where((m % 64) < 32, m + 32, m - 32)
    cb[sw, 1, m] = 1.0
    # 2: Tc diag mask, 3: zeros (mid), 4: Tw window mask -> contiguous seed
    p = np.arange(128)[:, None]
    c = np.arange(128)[None, :]
    cb[:, 2, :] = np.where(c >= p, 0.0, NEG)
    cb[:, 4, :] = np.where(c <= p, 0.0, NEG)
    # 10: cols 0:4 = msq stationary for even r, 4:8 odd r, 8:10 indq2 (k)
    cb[0:64, 10, 0] = 1.0 / D
    cb[64:128, 10, 1] = 1.0 / D
    cb[0:64, 10, 6] = 1.0 / D
    cb[64:128, 10, 7] = 1.0 / D
    cb[0:64, 10, 8] = 1.0 / D
    cb[64:128, 10, 9] = 1.0 / D

    # 5..8: ind014 per r: rsq4 row (2*(r%2)+gg) -> out gg rows, val 1/QK_SCALE
    for r in range(4):
        i = r % 2
        cb[2 * i, 5 + r, 0:64] = 1.0 / QK_SCALE
        cb[2 * i + 1, 5 + r, 64:128] = 1.0 / QK_SCALE
    # 9: indbk rows 0:2 (1/1.2), 10: indbg rows 0:2 (1.0), 11: ones row 0
    cb[0, 9, 0:64] = 1.0 / 1.2
    cb[1, 9, 64:128] = 1.0 / 1.2
    cb[0, 11, :] = 1.0
    epsb = np.full((128, 1), EPS, dtype=np.float32)
    return dict(cb16=cb.astype(BF16), cepsb=epsb)


def _prep_core_inputs(x, ve3, cosb, sinbw, Wq, Wk, Wv, Wo, Wg, consts, b, s):
    g0, g1 = 2 * s, 2 * s + 1
    bf = lambda a: np.ascontiguousarray(a).astype(BF16)
    xt = bf(x[b].T)

    Wq4 = Wq.reshape(HKV, REP, D, C)
    wq_rows = np.concatenate([Wq4[g, r] for r in range(REP) for g in (g0, g1)],
                             axis=0)                       # (512, C)
    wq_full = bf(wq_rows.T)                                # (C, 512)
    Wk3 = Wk.reshape(HKV, D, C)
    wk = bf(np.concatenate([Wk3[g0], Wk3[g1]], axis=0).T)
    Wv3 = Wv.reshape(HKV, D, C)
    wv = bf(np.concatenate([Wv3[g0], Wv3[g1]], axis=0).T)

    Wo4 = Wo.reshape(C, HKV, REP, D)
    wo_cols = np.concatenate([Wo4[:, g, r, :] for r in range(REP)
                              for g in (g0, g1)], axis=1)  # (C, 512)
    wo = bf(wo_cols.T)                                     # (512, C)

    wgn = np.zeros((16, 2), dtype=np.float32)
    wgn[0:GATE_CH, 0] = -Wg[g0]
    wgn[0:GATE_CH, 1] = -Wg[g1]

    ve4 = ve3[b].reshape(T, HKV, D)
    vet2 = np.concatenate([ve4[:, g0, :], ve4[:, g1, :]], axis=1)  # (T, 128)
    vet = bf(vet2.reshape(8, 128, 128).transpose(1, 0, 2))  # (128, 8, 128)

    d = dict(xt=xt, wqa=np.ascontiguousarray(wq_full[:, 0:256]),
             wqb=np.ascontiguousarray(wq_full[:, 256:512]),
             wk=wk, wv=wv, wo=wo, wgn=wgn.astype(BF16),
             vet=vet, cosb=cosb, sinbw=sinbw)
    d.update(consts)
    return d


def kernel(x, ve, cos, sin, Wq, Wk, Wv, Wo, Wg, window_size):
    from concourse.bass_utils import run_bass_kernel_spmd

    assert int(window_size) == WINDOW
    x = np.asarray(x, dtype=np.float32)
    ve = np.asarray(ve, dtype=np.float32)
    Wq = np.asarray(Wq, dtype=np.float32)
    Wk = np.asarray(Wk, dtype=np.float32)
    Wv = np.asarray(Wv, dtype=np.float32)
    Wo = np.asarray(Wo, dtype=np.float32)
    Wg = np.asarray(Wg, dtype=np.float32)
    c = np.asarray(cos, dtype=np.float32).reshape(T, D // 2)   # (T, 32)
    sn = np.asarray(sin, dtype=np.float32).reshape(T, D // 2)

    cosb = np.ascontiguousarray(np.tile(c.T, (4, 1))).astype(BF16)
    sinbw = np.ascontiguousarray(
        np.concatenate([-sn.T, sn.T, -sn.T, sn.T], axis=0)).astype(BF16)
    ve3 = 3.0 * ve
    consts = _const_inputs()

    if "nc" not in _CACHE:
        _CACHE["nc"] = _build_program()
    nc = _CACHE["nc"]

    in_maps = []
    for core in range(NCORES):
        b, s = core // 2, core % 2
        in_maps.append(_prep_core_inputs(x, ve3, cosb, sinbw,
                                         Wq, Wk, Wv, Wo, Wg, consts, b, s))

    res = run_bass_kernel_spmd(nc, in_maps, core_ids=list(range(NCORES)))
    out = np.empty((B, T, C), dtype=np.float32)
    for b in range(B):
        acc = (res.results[2 * b]["out_t"].astype(np.float32)
               + res.results[2 * b + 1]["out_t"].astype(np.float32))
        out[b] = acc.T
    return out
